# revision 31
# baseline (speedup 1.0000x reference)
"""Distributed Trainium2 Bass kernel for nn_Attention (GQA attention + LoRA + RoPE).

Sharding: tensor-parallel over heads across 8 NeuronCores.
  - core c owns Q heads 4c..4c+3 and KV head c (GQA group).
  - wq/wk/wv column-sharded; wo COLUMN-sharded (each core computes a
    512-column slice of the output over the full 4096 contraction, fed by an
    AllGather of all cores' per-head attention outputs).
  - LoRA is folded into wq/wv on the host (x@wq + (x@A)@B == x@(wq + A@B)).
  - 1/sqrt(HD) folded into wq.
  - RoPE pair permutation folded into wq/wk column order: within each head the
    even dims come first, odd dims second, so on-device RoPE is plain
    elementwise math on partition halves.

Everything the device computes is bf16-in/f32-accumulate.

v3 performance notes (vs the 432us v2):
  The chip runs GPIO-power-throttled to 13/16 (~1.95GHz PE) for ~90% of the
  kernel, so v3 attacks streamed-column count and non-PE stalls:
  - softmax denominator no longer uses per-st ones-matmuls: pr tiles are
    summed on DVE (bf16) and a single ones-matmul per (head, chunk)
    broadcasts the denominator (-28.7K PE columns).
  - QK head-pairs pack into one 2-bank PSUM tile with the causal-narrowed
    spans adjacent, so ONE exp ACTIVATE covers two heads with no wasted
    columns (scalar exp time 58us -> 22us).
  - causal masking via a DVE multiply with a constant lower-tri mask
    (gpsimd affine_select stalled the PE behind collective triggers).
  - PV matmuls lag QK by one st block so exp latency never stalls the PE.
  - proj quarter 0 runs K/V matmuls 8 k-tiles ahead of Q matmuls and the
    weight stream is reordered to match (kills a 7.5us HBM-starvation stall).
  - PSUM: 2-bank pair tiles (kv / q23 in proj, QK pairs in attention,
    2 of 4 wo accumulators) + 4 single banks, exactly filling 8 banks.
"""

import sys
import types

import numpy as np
import ml_dtypes

import concourse.bass as bass
from concourse import bacc
import concourse.mybir as mybir
import concourse.tile as tile
from concourse.bass_utils import run_bass_kernel_spmd
from concourse.masks import make_identity


def _ensure_axon_hooks():
    """run_bass_kernel_spmd(trace=True) imports antenv.axon_hooks, which some
    images lack; install a no-op shim so a BASS_TRACE env var can't crash us."""
    try:
        import antenv
    except ImportError:
        return
    if "antenv.axon_hooks" in sys.modules:
        return
    try:
        from antenv import axon_hooks  # noqa: F401
        return
    except ImportError:
        pass
    mod = types.ModuleType("antenv.axon_hooks")
    mod._hook = None
    mod.set_axon_ntff_profile_hook = lambda h: setattr(mod, "_hook", h)
    mod.get_axon_ntff_profile_hook = lambda: mod._hook
    sys.modules["antenv.axon_hooks"] = mod
    antenv.axon_hooks = mod


_ensure_axon_hooks()

B, S, D = 2, 1024, 4096
H, KVH, HD = 32, 8, 128
NCORES = 8
HPC = H // NCORES            # 4 q heads per core
QCOLS = HPC * HD             # 512
T = B * S                    # 2048
P = 128
KT = D // P                  # 32 k tiles
NQ = 4                       # token quarters (512 tokens each)
QW = T // NQ                 # 512
SQC = 2                      # sq chunks per batch
STB = S // P                 # 8 st blocks per batch
XTC = 4                      # k-tiles per xt DMA chunk
AGC = 2                      # k-tiles per allgather-readback DMA chunk
QLAG = 8                     # quarter-0 K/V lead over Q, in k-tiles

FP32 = mybir.dt.float32
BF16 = mybir.dt.bfloat16
EXP = mybir.ActivationFunctionType.Exp

_COMPILED = {}
LAST_RESULTS = None


def _st_info(variant, sqc):
    """st blocks contributing to sq chunk sqc, as (st, r, sel):
    r = first needed column within the 512-wide chunk (0 for full width),
    sel = start of the 128-wide diagonal span needing triangular zeroing
    (None if the block is fully below the diagonal / no mask)."""
    out = []
    for st in range(STB):
        if variant == "causal":
            rd = st * P - sqc * QW
            if rd >= QW:
                continue  # fully masked
            if rd >= 0:
                out.append((st, rd, rd))
            else:
                out.append((st, 0, None))
        else:
            out.append((st, 0, None))
    return out


def _build(variant):
    nc = bacc.Bacc(None)

    # xt packed quarter-major: [:, qx, k, :] is per-partition contiguous 4KB
    # per 4-ktile chunk, so xt chunk DMAs run at large-descriptor efficiency.
    xt_e = nc.declare_dram_parameter("xt", [P, NQ, KT, QW], BF16, isOutput=False)
    wq_e = nc.declare_dram_parameter("wq", [P, KT, QCOLS], BF16, isOutput=False)
    wk_e = nc.declare_dram_parameter("wk", [P, KT, HD], BF16, isOutput=False)
    wv_e = nc.declare_dram_parameter("wv", [P, KT, HD], BF16, isOutput=False)
    wo_e = nc.declare_dram_parameter("wo", [P, KT, QCOLS], BF16, isOutput=False)
    # cos: [c; c] duplicated halves.  sin: [s; -s] (negated bottom half).
    cos_e = nc.declare_dram_parameter("cos", [P, T], BF16, isOutput=False)
    sin_e = nc.declare_dram_parameter("sin", [P, T], BF16, isOutput=False)
    if variant == "general":
        mk_e = nc.declare_dram_parameter("mk", [P, STB, S], BF16, isOutput=False)
    out_e = nc.declare_dram_parameter("out", [QCOLS, T], BF16, isOutput=True)

    with tile.TileContext(nc) as tc:
        with (
            tc.tile_pool(name="wpool", bufs=1) as wpool,
            tc.tile_pool(name="cst", bufs=1) as cst,
            tc.tile_pool(name="persist", bufs=1) as persist,
            tc.tile_pool(name="xt", bufs=10) as xtp,
            tc.tile_pool(name="ev", bufs=4) as evp,
            tc.tile_pool(name="rt", bufs=3) as rtp,
            tc.tile_pool(name="probs", bufs=7 if variant != "general" else 20) as prp,
            tc.tile_pool(name="dacc", bufs=4) as dap,
            tc.tile_pool(name="misc", bufs=3) as mip,
            tc.tile_pool(name="ag", bufs=12) as agp,
            tc.tile_pool(name="ow", bufs=8) as owp,
            tc.tile_pool(name="ps2", bufs=2, space="PSUM") as ps2,
            tc.tile_pool(name="ps1", bufs=4, space="PSUM") as ps1,
            tc.tile_pool(name="dram", bufs=1, space="DRAM") as dram,
        ):
            # ---- resident weights / constants ----
            # wq_sb doubles as wo storage: wo is DMA'd over it after proj
            # quarter 3's last wq read (Tile WAR tracking orders this).
            wq_sb = wpool.tile([P, KT, QCOLS], BF16, name="wq_sb")
            wk_sb = wpool.tile([P, KT, HD], BF16, name="wk_sb")
            wv_sb = wpool.tile([P, KT, HD], BF16, name="wv_sb")
            cos_sb = wpool.tile([P, T], BF16, name="cos_sb")
            sin_sb = wpool.tile([P, T], BF16, name="sin_sb")
            if variant == "general":
                mk_sb = wpool.tile([P, STB, S], BF16, name="mk_sb")

            ident = cst.tile([P, P], BF16, name="ident")
            make_identity(nc, ident)
            ones_sq = cst.tile([P, P], BF16, name="ones_sq")
            nc.vector.memset(ones_sq[:], 1.0)
            # additive causal mask for a diagonal 128x128 span:
            # tri_neg[p, j] = 0 if p <= j else -1e9 (future keys killed
            # pre-exp, so no post-exp masking op is needed anywhere).
            zeros_sq = cst.tile([P, P], BF16, name="zeros_sq")
            nc.vector.memset(zeros_sq[:], 0.0)
            tri_neg = cst.tile([P, P], BF16, name="tri_neg")
            nc.gpsimd.affine_select(
                out=tri_neg[:], in_=zeros_sq[:],
                compare_op=mybir.AluOpType.is_ge, fill=-1e9,
                base=0, channel_multiplier=-1, pattern=[[1, P]])

            # ---- persistent activations ----
            q_rot = [[persist.tile([P, S], BF16, name=f"q{h}_{b}")
                      for b in range(B)] for h in range(HPC)]
            k_rot = [persist.tile([P, S], BF16, name=f"k{b}") for b in range(B)]
            v_sb = [persist.tile([P, STB, P], BF16, name=f"v{b}") for b in range(B)]
            attn = [[persist.tile([P, S], BF16, name=f"attn{h}_{b}")
                     for b in range(B)] for h in range(HPC)]

            # per-(batch, sq-half) gather buffers: two smaller collectives per
            # batch, each fired as soon as its attention chunk finishes --
            # spreads collective traffic and halves skew exposure.
            ag_in = [[dram.tile([HPC * P, QW], BF16, name=f"agin{b}_{c}")
                      for c in range(SQC)] for b in range(B)]
            ag_out = [[dram.tile([H * P, QW], BF16, addr_space="Shared",
                                 name=f"agout{b}_{c}") for c in range(SQC)]
                      for b in range(B)]

            def rope(dst, dst_off, src_bf, qoff):
                """RoPE on split layout (a=0:64, b=64:128), 4 DVE ops.
                p1 = [a*c; b*c];  p2sw = [b*(-s); a*s] computed directly with
                cross-partition reads (sin table already holds [s; -s]);
                dst = p1 + p2sw = [a*c - b*s; a*s + b*c]."""
                c = cos_sb[:, qoff:qoff + QW]
                p1 = rtp.tile([P, QW], BF16, name="p1")
                p2sw = rtp.tile([P, QW], BF16, name="p2sw")
                nc.vector.tensor_mul(p1[:], src_bf[:], c)
                nc.vector.tensor_mul(p2sw[0:64, :], src_bf[64:128, :],
                                     sin_sb[64:128, qoff:qoff + QW])
                nc.vector.tensor_mul(p2sw[64:128, :], src_bf[0:64, :],
                                     sin_sb[0:64, qoff:qoff + QW])
                nc.vector.tensor_add(dst[:, dst_off:dst_off + QW], p1[:], p2sw[:])

            def load_weights():
                """Stream projection weights + rope tables on the scalar
                (HWDGE) queue, ordered by first consumption under the
                quarter-0 schedule (K/V leading Q by QLAG k-tiles), so the
                sync queue only carries the xt stream and no weight arrives
                later than the matmul that needs it."""
                dma = nc.scalar.dma_start

                def tables(qx):
                    # rope tables ride the otherwise-idle gpsimd queue so the
                    # scalar queue carries only the weight stream.
                    toff = qx * QW
                    nc.gpsimd.dma_start(cos_sb[:, toff:toff + QW],
                                        cos_e[:, toff:toff + QW])
                    nc.gpsimd.dma_start(sin_sb[:, toff:toff + QW],
                                        sin_e[:, toff:toff + QW])

                dma(wk_sb[:, 0:2, :], wk_e[:, 0:2, :])
                dma(wv_sb[:, 0:2, :], wv_e[:, 0:2, :])
                dma(wk_sb[:, 2:8, :], wk_e[:, 2:8, :])
                dma(wv_sb[:, 2:8, :], wv_e[:, 2:8, :])
                dma(wk_sb[:, 8:12, :], wk_e[:, 8:12, :])
                dma(wv_sb[:, 8:12, :], wv_e[:, 8:12, :])
                dma(wq_sb[:, 0:4, :], wq_e[:, 0:4, :])
                dma(wk_sb[:, 12:16, :], wk_e[:, 12:16, :])
                dma(wv_sb[:, 12:16, :], wv_e[:, 12:16, :])
                dma(wq_sb[:, 4:8, :], wq_e[:, 4:8, :])
                dma(wk_sb[:, 16:20, :], wk_e[:, 16:20, :])
                dma(wv_sb[:, 16:20, :], wv_e[:, 16:20, :])
                dma(wq_sb[:, 8:12, :], wq_e[:, 8:12, :])
                dma(wk_sb[:, 20:24, :], wk_e[:, 20:24, :])
                dma(wv_sb[:, 20:24, :], wv_e[:, 20:24, :])
                dma(wq_sb[:, 12:16, :], wq_e[:, 12:16, :])
                dma(wq_sb[:, 16:20, :], wq_e[:, 16:20, :])
                # the 12-ktile q tail consumes wq[20:32] before wk/wv[24:32]
                # are touched; tables are only needed at eviction time.
                dma(wq_sb[:, 20:24, :], wq_e[:, 20:24, :])
                dma(wq_sb[:, 24:28, :], wq_e[:, 24:28, :])
                dma(wq_sb[:, 28:KT, :], wq_e[:, 28:KT, :])
                dma(wk_sb[:, 24:KT, :], wk_e[:, 24:KT, :])
                dma(wv_sb[:, 24:KT, :], wv_e[:, 24:KT, :])
                tables(0)
                tables(1)
                tables(2)
                tables(3)
                if variant == "general":
                    dma(mk_sb[:], mk_e[:])

            def wo_load():
                """Overwrite wq_sb with wo (WAR-ordered after the last wq read,
                i.e. streams during attention_batch(1)).  On the scalar queue so
                the sync queue only carries agt readback for wo_batch(0)."""
                for c in range(4):
                    nc.scalar.dma_start(wq_sb[:, 8 * c:8 * c + 8, :],
                                        wo_e[:, 8 * c:8 * c + 8, :])

            def proj_quarter(qx):
                b, boff = qx // 2, (qx % 2) * QW
                toff = qx * QW
                # psums: kv pair (k | v), q0, q1 single banks, q23 pair.
                kv_ps = ps2.tile([P, 2 * QW], FP32, name="kv_ps", tag="pair")
                q01 = [ps1.tile([P, QW], FP32, name="q01", tag="ps")
                       for _ in range(2)]
                q23_ps = ps2.tile([P, 2 * QW], FP32, name="q23_ps", tag="pair")

                def mm(mb, k, xt, start, stop):
                    if mb == 0:
                        w, dst = wk_sb[:, k, :], kv_ps[:, 0:QW]
                    elif mb == 1:
                        w, dst = wv_sb[:, k, :], kv_ps[:, QW:2 * QW]
                    elif mb < 4:
                        w = wq_sb[:, k, (mb - 2) * P:(mb - 1) * P]
                        dst = q01[mb - 2][:]
                    else:
                        w = wq_sb[:, k, (mb - 2) * P:(mb - 1) * P]
                        dst = q23_ps[:, (mb - 4) * QW:(mb - 3) * QW]
                    nc.tensor.matmul(dst, w, xt, start=start, stop=stop)

                def kvmm(k, xt):
                    mm(0, k, xt, k == 0, k == KT - 1)
                    mm(1, k, xt, k == 0, k == KT - 1)

                def qmm(k, xt, mbs=(2, 3, 4, 5)):
                    for mb in mbs:
                        mm(mb, k, xt, k == 0, k == KT - 1)

                evicted = []

                def evict_kv():
                    ke = evp.tile([P, QW], BF16, name="ke", tag="qe")
                    nc.scalar.copy(ke[:], kv_ps[:, 0:QW])
                    rope(k_rot[b], boff, ke, toff)
                    ve = evp.tile([P, QW], BF16, name="ve", tag="qe")
                    nc.scalar.copy(ve[:], kv_ps[:, QW:2 * QW])
                    evicted.append(ve)

                def evict_q(h):
                    qe = evp.tile([P, QW], BF16, name="qe", tag="qe")
                    if h < 2:
                        nc.scalar.copy(qe[:], q01[h][:])
                    else:
                        nc.scalar.copy(qe[:], q23_ps[:, (h - 2) * QW:(h - 1) * QW])
                    rope(q_rot[h][b], boff, qe, toff)

                tps = []

                def v_transposes():
                    ve = evicted[0]
                    for i in range(QW // P):
                        tp = ps1.tile([P, P], BF16, name="tp", tag="ps")
                        nc.tensor.transpose(tp[:], ve[:, i * P:(i + 1) * P],
                                            ident[:])
                        tps.append(tp)

                def v_copies(eng=None):
                    for i, tp in enumerate(tps):
                        st = (qx % 2) * 4 + i
                        if eng == "vector":
                            nc.vector.tensor_copy(v_sb[b][:, st, :], tp[:])
                        else:
                            nc.scalar.copy(v_sb[b][:, st, :], tp[:])

                xts = {}
                if qx == 0:
                    # K/V lead Q by QLAG k-tiles so wq k-tiles have an extra
                    # ~12us of HBM-stream slack during the cold start.
                    chunks = [1, 1, 2, 2, 2] + [XTC] * 6
                    k0 = 0
                    qptr = 0
                    for cw in chunks:
                        xt4 = xtp.tile([P, XTC, QW], BF16, name="xt4")
                        nc.sync.dma_start(xt4[:, 0:cw, :],
                                          xt_e[:, qx, k0:k0 + cw, :])
                        for j in range(cw):
                            xts[k0 + j] = xt4[:, j, :]
                            kvmm(k0 + j, xts[k0 + j])
                        k0 += cw
                        # cap the catch-up so a 12-ktile Q tail remains: its
                        # mb-major groups are long enough that each head's
                        # eviction + rope completes during the next group.
                        while qptr <= min(k0 - 1 - QLAG, KT - 13):
                            qmm(qptr, xts.pop(qptr))
                            qptr += 1
                    evict_kv()
                    # Q tail mb-major: early heads stop first so their
                    # evictions + ropes overlap the remaining matmuls.
                    for mb in (2, 3, 4, 5):
                        for kq in range(qptr, KT):
                            mm(mb, kq, xts[kq], kq == 0, kq == KT - 1)
                        evict_q(mb - 2)
                        if mb == 3:
                            v_transposes()
                    v_copies()
                else:
                    chunks = [XTC] * (KT // XTC)
                    k0 = 0
                    tail = []
                    for ci, cw in enumerate(chunks):
                        xt4 = xtp.tile([P, XTC, QW], BF16, name="xt4")
                        nc.sync.dma_start(xt4[:, 0:cw, :],
                                          xt_e[:, qx, k0:k0 + cw, :])
                        if ci < len(chunks) - 3:
                            for j in range(cw):
                                k = k0 + j
                                kvmm(k, xt4[:, j, :])
                                qmm(k, xt4[:, j, :])
                        else:
                            tail += [(k0 + j, xt4[:, j, :]) for j in range(cw)]
                        k0 += cw
                    # 12-ktile tail, mb-major ordered K, q0..q3, V: each
                    # eviction + rope (a ~2.7us scalar+DVE chain) completes
                    # during the following ~2.8us matmul group, so every rope
                    # the next attention phase needs is ready when its first
                    # QK issues.  V runs last: its eviction chain (transposes
                    # + v_sb copies on DVE) only gates the PV matmuls ~2.5us
                    # into the attention phase.
                    for k, xt in tail:
                        mm(0, k, xt, k == 0, k == KT - 1)
                    ke = evp.tile([P, QW], BF16, name="ke", tag="qe")
                    nc.scalar.copy(ke[:], kv_ps[:, 0:QW])
                    rope(k_rot[b], boff, ke, toff)
                    for mb in (2, 3, 4, 5):
                        for k, xt in tail:
                            mm(mb, k, xt, k == 0, k == KT - 1)
                        evict_q(mb - 2)
                    for k, xt in tail:
                        mm(1, k, xt, k == 0, k == KT - 1)
                    ve = evp.tile([P, QW], BF16, name="ve", tag="qe")
                    nc.scalar.copy(ve[:], kv_ps[:, QW:2 * QW])
                    evicted.append(ve)
                    v_transposes()
                    v_copies(eng="vector")

            def attention_batch(b, sqcs=tuple(range(SQC))):
                if variant == "general":
                    attention_batch_general(b, sqcs)
                    return
                for sqc in sqcs:
                    sq0 = sqc * QW
                    stl = _st_info(variant, sqc)
                    # interleave full-width and narrowed st blocks so the
                    # scalar engine's exp stream (cost ~ width) never falls
                    # behind the PE during a run of full-width blocks; keep a
                    # full-width block first (the dacc copy needs full width).
                    full = [e for e in stl if e[1] == 0]
                    narrow = sorted([e for e in stl if e[1] > 0],
                                    key=lambda e: -e[1])
                    stl = []
                    while full or narrow:
                        if full:
                            stl.append(full.pop(0))
                        if narrow:
                            stl.append(narrow.pop())
                    n = len(stl)
                    pairs = ((0, 1), (2, 3))
                    pso = [ps1.tile([P, QW], FP32, name="pso", tag="ps")
                           for _ in range(HPC)]
                    dacc = [dap.tile([P, QW], BF16, name="dacc")
                            for _ in range(HPC)]
                    prs = []

                    def pv(i):
                        st, r, sel = stl[i]
                        for pi, hh in enumerate(pairs):
                            pr = prs[i][pi]
                            for sl, h in enumerate(hh):
                                src = pr[:, r:QW] if sl == 0 \
                                    else pr[:, QW:2 * QW - r]
                                nc.tensor.matmul(pso[h][:, r:QW],
                                                 v_sb[b][:, st, :], src,
                                                 start=(i == 0),
                                                 stop=(i == n - 1))

                    def finish_head(h):
                        # denominator broadcast + normalize + ship to gather
                        psd = ps2.tile([P, 2 * QW], FP32, name="psd",
                                       tag="pair")
                        nc.tensor.matmul(psd[:, 0:QW], ones_sq[:], dacc[h][:],
                                         start=True, stop=True)
                        rb = mip.tile([P, QW], FP32, name="rb")
                        nc.vector.reciprocal_approx_fast(rb[:], psd[:, 0:QW])
                        nc.vector.tensor_mul(attn[h][b][:, sq0:sq0 + QW],
                                             pso[h][:], rb[:])
                        nc.gpsimd.dma_start(
                            ag_in[b][sqc][h * P:(h + 1) * P, :],
                            attn[h][b][:, sq0:sq0 + QW])

                    prs.extend([None, None] for _ in range(n))

                    def emit_pair(i, pi):
                        st, r, sel = stl[i]
                        assert sel is None or sel == r
                        hh = pairs[pi]
                        # both heads of the pair in one 2-bank psum; the
                        # causal-narrowed spans pack adjacently so one exp
                        # covers both heads with no wasted columns:
                        # h0 at [r:QW], h1 at [QW:2QW-r].
                        pss = ps2.tile([P, 2 * QW], FP32, name="pss",
                                       tag="pair")
                        kst = k_rot[b][:, st * P:(st + 1) * P]
                        if sel is not None:
                            # additive causal mask preloaded into the
                            # diagonal span of each head's score bank:
                            # exp then yields exact zeros for future
                            # keys, keeping the PV chain PE+ACT-only.
                            nc.tensor.matmul(pss[:, r:r + P], ident[:],
                                             tri_neg[:],
                                             start=True, stop=False)
                            nc.tensor.matmul(pss[:, QW:QW + P], ident[:],
                                             tri_neg[:],
                                             start=True, stop=False)
                        nc.tensor.matmul(
                            pss[:, r:QW], kst,
                            q_rot[hh[0]][b][:, sq0 + r:sq0 + QW],
                            start=(sel is None), stop=True)
                        nc.tensor.matmul(
                            pss[:, QW:2 * QW - r], kst,
                            q_rot[hh[1]][b][:, sq0 + r:sq0 + QW],
                            start=(sel is None), stop=True)
                        pr = prp.tile([P, 2 * QW], BF16, name="pr",
                                      tag="pr")
                        if i == 0:
                            # per-head exps at the chunk's first block: each
                            # starts one QK earlier and is half as long, so
                            # the second block's QKs (which reuse this psum
                            # slot) stall ~1us less while the exp pipeline
                            # fills.
                            nc.scalar.activation(pr[:, r:QW],
                                                 pss[:, r:QW], EXP)
                            nc.scalar.activation(pr[:, QW:2 * QW - r],
                                                 pss[:, QW:2 * QW - r], EXP)
                        else:
                            nc.scalar.activation(pr[:, r:2 * QW - r],
                                                 pss[:, r:2 * QW - r], EXP)
                        # denominator partial sums on DVE (bf16); only
                        # consumed by the ones-matmul at chunk end, so
                        # DVE lag never stalls the PE.  The last two blocks'
                        # adds are deferred and interleaved per-head with
                        # finish_head so recip(h0) (whose completion
                        # releases the next proj phase's PSUM banks via
                        # WAR) runs as early as possible.
                        if i < n - 2:
                            for sl, h in enumerate(hh):
                                src = pr[:, r:QW] if sl == 0 \
                                    else pr[:, QW:2 * QW - r]
                                if i == 0:
                                    nc.vector.tensor_copy(dacc[h][:], src)
                                else:
                                    nc.vector.tensor_add(
                                        dacc[h][:, r:QW],
                                        dacc[h][:, r:QW], src)
                        prs[i][pi] = pr

                    for i in range(n):
                        emit_pair(i, 0)
                        emit_pair(i, 1)
                        if i >= 1:
                            pv(i - 1)
                    pv(n - 1)
                    for h in range(HPC):
                        for i in (n - 2, n - 1):
                            st, r, sel = stl[i]
                            pr = prs[i][h // 2]
                            src = pr[:, r:QW] if h % 2 == 0 \
                                else pr[:, QW:2 * QW - r]
                            nc.vector.tensor_add(dacc[h][:, r:QW],
                                                 dacc[h][:, r:QW], src)
                        finish_head(h)

            def attention_batch_general(b, sqcs):
                for sqc in sqcs:
                    sq0 = sqc * QW
                    stl = _st_info(variant, sqc)
                    n = len(stl)
                    for h in range(HPC):
                        prtiles = []
                        for st, r, sel in stl:
                            pss = ps1.tile([P, QW], FP32, name="pss", tag="ps")
                            nc.tensor.matmul(pss[:], ident[:],
                                             mk_sb[:, st, sq0:sq0 + QW],
                                             start=True, stop=False)
                            nc.tensor.matmul(
                                pss[:], k_rot[b][:, st * P:(st + 1) * P],
                                q_rot[h][b][:, sq0:sq0 + QW],
                                start=False, stop=True)
                            pr = prp.tile([P, QW], BF16, name="pr", tag="pr")
                            nc.scalar.activation(pr[:], pss[:], EXP)
                            prtiles.append(pr)
                        pso = ps2.tile([P, 2 * QW], FP32, name="psog",
                                       tag="pair")
                        for i, (st, r, sel) in enumerate(stl):
                            pr = prtiles[i]
                            nc.tensor.matmul(pso[:, 0:QW], v_sb[b][:, st, :],
                                             pr[:],
                                             start=(i == 0), stop=(i == n - 1))
                            nc.tensor.matmul(pso[:, QW:2 * QW], ones_sq[:],
                                             pr[:],
                                             start=(i == 0), stop=(i == n - 1))
                        rb = mip.tile([P, QW], FP32, name="rb")
                        nc.vector.reciprocal_approx_fast(rb[:],
                                                         pso[:, QW:2 * QW])
                        nc.vector.tensor_mul(attn[h][b][:, sq0:sq0 + QW],
                                             pso[:, 0:QW], rb[:])
                        nc.gpsimd.dma_start(
                            ag_in[b][sqc][h * P:(h + 1) * P, :],
                            attn[h][b][:, sq0:sq0 + QW])

            def gather_batch(b, c):
                nc.gpsimd.collective_compute(
                    "AllGather", mybir.AluOpType.bypass,
                    ins=[ag_in[b][c][:].opt()],
                    outs=[ag_out[b][c][:].opt()],
                    replica_groups=[list(range(NCORES))],
                )

            def wo_chunk(b, nch, last):
                """wo matmuls for one (batch, sq-chunk): full 4096-contraction
                over 512 tokens.  Processing sq-chunks serially (not
                interleaved) means chunk 0's matmuls only need that chunk's
                AllGather -- chunk 1's gather (the last collective for the
                batch) gets an extra ~35us of slack before first use."""
                ag_r = ag_out[b][nch].rearrange("(k p) t -> p k t", p=P)
                psw_pair = ps2.tile([P, 2 * QW], FP32, name="psw_pair",
                                    tag="pair")
                psw_s = [ps1.tile([P, QW], FP32, name="psw", tag="ps")
                         for _ in range(2)]

                def psw(mb):
                    if mb < 2:
                        return psw_pair[:, mb * QW:(mb + 1) * QW]
                    return psw_s[mb - 2][:]

                nchk = KT // AGC
                wtail = []
                for kc in range(nchk):
                    agt = agp.tile([P, AGC, QW], BF16, name="agt")
                    nc.sync.dma_start(agt[:],
                                      ag_r[:, kc * AGC:(kc + 1) * AGC, :])
                    if kc < nchk - 2:
                        for j in range(AGC):
                            k = kc * AGC + j
                            for mb in range(4):
                                w = wq_sb[:, k, mb * P:(mb + 1) * P]
                                nc.tensor.matmul(
                                    psw(mb), w, agt[:, j, :],
                                    start=(k == 0), stop=False)
                    else:
                        wtail += [(kc * AGC + j, agt) for j in range(AGC)]
                # last two chunks mb-major so early mb groups stop several us
                # before the end and their evictions + out DMAs overlap the
                # remaining matmuls.
                tcol = b * S + nch * QW
                for mb in range(4):
                    for k, agt in wtail:
                        w = wq_sb[:, k, mb * P:(mb + 1) * P]
                        nc.tensor.matmul(
                            psw(mb), w, agt[:, k % AGC, :],
                            start=False, stop=(k == KT - 1))
                    if mb == 3 and last:
                        # split the very last eviction in half across
                        # engines/queues so its copy + DMA pipeline
                        # instead of serializing after the final matmul.
                        hw = QW // 2
                        for hf in range(2):
                            ow = owp.tile([P, hw], BF16, name="owh",
                                          tag="owh")
                            src = psw(mb)[:, hf * hw:(hf + 1) * hw]
                            if hf == 0:
                                nc.vector.tensor_copy(ow[:], src)
                                dma = nc.sync.dma_start
                            else:
                                nc.scalar.copy(ow[:], src)
                                dma = nc.scalar.dma_start
                            dma(out_e[mb * P:(mb + 1) * P,
                                      tcol + hf * hw:tcol + (hf + 1) * hw],
                                ow[:])
                        continue
                    ow = owp.tile([P, QW], BF16, name="ow")
                    if mb % 2 == 0:
                        nc.scalar.copy(ow[:], psw(mb))
                        dma = nc.scalar.dma_start
                    else:
                        nc.vector.tensor_copy(ow[:], psw(mb))
                        dma = nc.sync.dma_start
                    dma(out_e[mb * P:(mb + 1) * P, tcol:tcol + QW], ow[:])

            def wo_batch(b):
                for nch in range(SQC):
                    wo_chunk(b, nch, last=(b == B - 1 and nch == SQC - 1))

            # ---- timeline ----
            # attention sq-chunks interleave between proj quarters: chunk s0 of
            # batch b only needs that batch's first token quarter. This fires
            # gather(0) earlier and gives the xt/weight streams HBM-quiet
            # windows (attention phases do no HBM traffic) to get ahead.
            load_weights()
            proj_quarter(0)
            attention_batch(0, (0,))
            gather_batch(0, 0)
            proj_quarter(1)
            attention_batch(0, (1,))
            gather_batch(0, 1)
            proj_quarter(2)
            attention_batch(1, (0,))
            gather_batch(1, 0)
            proj_quarter(3)
            wo_load()
            attention_batch(1, (1,))
            gather_batch(1, 1)
            wo_batch(0)
            wo_batch(1)

    nc.compile()
    return nc


def _get_compiled(variant):
    if variant not in _COMPILED:
        _COMPILED[variant] = _build(variant)
    return _COMPILED[variant]


def _detect_variant(mask2d):
    if not np.any(mask2d):
        return "nomask"
    tril = np.tril(mask2d)
    if not np.any(tril):
        iu = np.triu_indices(S, 1)
        if np.all(mask2d[iu] <= -1e8):
            return "causal"
    return "general"


def _pack_kt(w):
    """[R*128, N] -> [128, R, N] so that [:, k, :] is rows k*128..k*128+127."""
    return np.ascontiguousarray(w.reshape(w.shape[0] // P, P, -1).transpose(1, 0, 2))


def kernel(x, wq, wk, wv, wo, lora_q_a, lora_q_b, lora_v_a, lora_v_b,
           freqs_cos, freqs_sin, mask, start_pos=0, **_):
    global LAST_RESULTS
    bf = ml_dtypes.bfloat16
    x = np.asarray(x, np.float32)
    wq = np.asarray(wq, np.float32)
    wk = np.asarray(wk, np.float32)
    wv = np.asarray(wv, np.float32)
    wo = np.asarray(wo, np.float32)
    lora_q_a = np.asarray(lora_q_a, np.float32)
    lora_q_b = np.asarray(lora_q_b, np.float32)
    lora_v_a = np.asarray(lora_v_a, np.float32)
    lora_v_b = np.asarray(lora_v_b, np.float32)
    cos = np.asarray(freqs_cos, np.float32)
    sin = np.asarray(freqs_sin, np.float32)
    mask2d = np.asarray(mask, np.float32).reshape(S, S)

    variant = _detect_variant(mask2d)
    nc = _get_compiled(variant)

    # fold LoRA + scale; permute rope pairs (evens then odds within each head)
    wq_eff = (wq + lora_q_a @ lora_q_b) * np.float32(1.0 / np.sqrt(HD))
    wv_eff = wv + lora_v_a @ lora_v_b
    perm = np.concatenate([np.arange(0, HD, 2), np.arange(1, HD, 2)])
    qperm = (np.arange(H)[:, None] * HD + perm[None, :]).reshape(-1)
    kperm = (np.arange(KVH)[:, None] * HD + perm[None, :]).reshape(-1)
    wq_eff = wq_eff[:, qperm]
    wk_p = wk[:, kperm]

    xt = np.ascontiguousarray(x.reshape(T, D).T)        # [4096, 2048]
    # [128, KT, T] -> quarter-major [128, NQ, KT, QW] (contiguous per chunk)
    xt_p = np.ascontiguousarray(
        _pack_kt(xt).reshape(P, KT, NQ, QW).transpose(0, 2, 1, 3)).astype(bf)
    c64 = np.tile(cos.T, (1, B))                        # [64, 2048]
    s64 = np.tile(sin.T, (1, B))
    cosT = np.concatenate([c64, c64], axis=0).astype(bf)   # [c; c]
    sinT = np.concatenate([s64, -s64], axis=0).astype(bf)  # [s; -s]

    if variant == "general":
        maskT = np.ascontiguousarray(mask2d.T)          # [st, sq]
        mk = _pack_kt(maskT).astype(bf)                 # [128, 8, 1024]
    else:
        mk = None

    in_maps = []
    for c in range(NCORES):
        im = {
            "xt": xt_p,
            "wq": _pack_kt(wq_eff[:, c * QCOLS:(c + 1) * QCOLS]).astype(bf),
            "wk": _pack_kt(wk_p[:, c * HD:(c + 1) * HD]).astype(bf),
            "wv": _pack_kt(wv_eff[:, c * HD:(c + 1) * HD]).astype(bf),
            "wo": _pack_kt(wo[:, c * QCOLS:(c + 1) * QCOLS]).astype(bf),
            "cos": cosT,
            "sin": sinT,
        }
        if mk is not None:
            im["mk"] = mk
        in_maps.append(im)

    res = run_bass_kernel_spmd(nc, in_maps, core_ids=list(range(NCORES)))
    LAST_RESULTS = res
    outT = np.concatenate([res.results[c]["out"] for c in range(NCORES)], axis=0)
    return np.ascontiguousarray(outT.T).reshape(B, S, D).astype(np.float32)


# revision 36
# speedup vs baseline: 1.0156x; 1.0156x over previous
"""Distributed Trainium2 Bass kernel for nn_Attention (GQA attention + LoRA + RoPE).

Sharding: tensor-parallel over heads across 8 NeuronCores.
  - core c owns Q heads 4c..4c+3 and KV head c (GQA group).
  - wq/wk/wv column-sharded; wo COLUMN-sharded (each core computes a
    512-column slice of the output over the full 4096 contraction, fed by an
    AllGather of all cores' per-head attention outputs).
  - LoRA is folded into wq/wv on the host (x@wq + (x@A)@B == x@(wq + A@B)).
  - 1/sqrt(HD) folded into wq.
  - RoPE pair permutation folded into wq/wk column order: within each head the
    even dims come first, odd dims second, so on-device RoPE is plain
    elementwise math on partition halves.

Everything the device computes is bf16-in/f32-accumulate.

v3 performance notes (vs the 432us v2):
  The chip runs GPIO-power-throttled to 13/16 (~1.95GHz PE) for ~90% of the
  kernel, so v3 attacks streamed-column count and non-PE stalls:
  - softmax denominator no longer uses per-st ones-matmuls: pr tiles are
    summed on DVE (bf16) and a single ones-matmul per (head, chunk)
    broadcasts the denominator (-28.7K PE columns).
  - QK head-pairs pack into one 2-bank PSUM tile with the causal-narrowed
    spans adjacent, so ONE exp ACTIVATE covers two heads with no wasted
    columns (scalar exp time 58us -> 22us).
  - causal masking via a DVE multiply with a constant lower-tri mask
    (gpsimd affine_select stalled the PE behind collective triggers).
  - PV matmuls lag QK by one st block so exp latency never stalls the PE.
  - proj quarter 0 runs K/V matmuls 8 k-tiles ahead of Q matmuls and the
    weight stream is reordered to match (kills a 7.5us HBM-starvation stall).
  - PSUM: 2-bank pair tiles (kv / q23 in proj, QK pairs in attention,
    2 of 4 wo accumulators) + 4 single banks, exactly filling 8 banks.
"""

import sys
import types

import numpy as np
import ml_dtypes

import concourse.bass as bass
from concourse import bacc
import concourse.mybir as mybir
import concourse.tile as tile
from concourse.bass_utils import run_bass_kernel_spmd
from concourse.masks import make_identity


def _ensure_axon_hooks():
    """run_bass_kernel_spmd(trace=True) imports antenv.axon_hooks, which some
    images lack; install a no-op shim so a BASS_TRACE env var can't crash us."""
    try:
        import antenv
    except ImportError:
        return
    if "antenv.axon_hooks" in sys.modules:
        return
    try:
        from antenv import axon_hooks  # noqa: F401
        return
    except ImportError:
        pass
    mod = types.ModuleType("antenv.axon_hooks")
    mod._hook = None
    mod.set_axon_ntff_profile_hook = lambda h: setattr(mod, "_hook", h)
    mod.get_axon_ntff_profile_hook = lambda: mod._hook
    sys.modules["antenv.axon_hooks"] = mod
    antenv.axon_hooks = mod


_ensure_axon_hooks()

B, S, D = 2, 1024, 4096
H, KVH, HD = 32, 8, 128
NCORES = 8
HPC = H // NCORES            # 4 q heads per core
QCOLS = HPC * HD             # 512
T = B * S                    # 2048
P = 128
KT = D // P                  # 32 k tiles
NQ = 4                       # token quarters (512 tokens each)
QW = T // NQ                 # 512
SQC = 2                      # sq chunks per batch
STB = S // P                 # 8 st blocks per batch
XTC = 4                      # k-tiles per xt DMA chunk
AGC = 2                      # k-tiles per allgather-readback DMA chunk
QLAG = 8                     # quarter-0 K/V lead over Q, in k-tiles

FP32 = mybir.dt.float32
BF16 = mybir.dt.bfloat16
EXP = mybir.ActivationFunctionType.Exp

_COMPILED = {}
LAST_RESULTS = None


def _st_info(variant, sqc):
    """st blocks contributing to sq chunk sqc, as (st, r, sel):
    r = first needed column within the 512-wide chunk (0 for full width),
    sel = start of the 128-wide diagonal span needing triangular zeroing
    (None if the block is fully below the diagonal / no mask)."""
    out = []
    for st in range(STB):
        if variant == "causal":
            rd = st * P - sqc * QW
            if rd >= QW:
                continue  # fully masked
            if rd >= 0:
                out.append((st, rd, rd))
            else:
                out.append((st, 0, None))
        else:
            out.append((st, 0, None))
    return out


def _build(variant):
    nc = bacc.Bacc(None)

    # xt packed quarter-major: [:, qx, k, :] is per-partition contiguous 4KB
    # per 4-ktile chunk, so xt chunk DMAs run at large-descriptor efficiency.
    xt_e = nc.declare_dram_parameter("xt", [P, NQ, KT, QW], BF16, isOutput=False)
    wq_e = nc.declare_dram_parameter("wq", [P, KT, QCOLS], BF16, isOutput=False)
    wk_e = nc.declare_dram_parameter("wk", [P, KT, HD], BF16, isOutput=False)
    wv_e = nc.declare_dram_parameter("wv", [P, KT, HD], BF16, isOutput=False)
    wo_e = nc.declare_dram_parameter("wo", [P, KT, QCOLS], BF16, isOutput=False)
    # cos: [c; c] duplicated halves.  sin: [s; -s] (negated bottom half).
    cos_e = nc.declare_dram_parameter("cos", [P, T], BF16, isOutput=False)
    sin_e = nc.declare_dram_parameter("sin", [P, T], BF16, isOutput=False)
    if variant == "general":
        mk_e = nc.declare_dram_parameter("mk", [P, STB, S], BF16, isOutput=False)
    out_e = nc.declare_dram_parameter("out", [QCOLS, T], BF16, isOutput=True)

    with tile.TileContext(nc) as tc:
        with (
            tc.tile_pool(name="wpool", bufs=1) as wpool,
            tc.tile_pool(name="cst", bufs=1) as cst,
            tc.tile_pool(name="persist", bufs=1) as persist,
            tc.tile_pool(name="xt", bufs=10 if variant != "general" else 8) as xtp,
            tc.tile_pool(name="ev", bufs=4) as evp,
            tc.tile_pool(name="rt", bufs=3) as rtp,
            tc.tile_pool(name="probs", bufs=7 if variant != "general" else 20) as prp,
            tc.tile_pool(name="dacc", bufs=4) as dap,
            tc.tile_pool(name="misc", bufs=3) as mip,
            tc.tile_pool(name="ag", bufs=12 if variant != "general" else 6) as agp,
            tc.tile_pool(name="ow", bufs=8) as owp,
            tc.tile_pool(name="ps2", bufs=2, space="PSUM") as ps2,
            tc.tile_pool(name="ps1", bufs=4, space="PSUM") as ps1,
            tc.tile_pool(name="dram", bufs=1, space="DRAM") as dram,
        ):
            # ---- resident weights / constants ----
            # wq_sb doubles as wo storage: wo is DMA'd over it after proj
            # quarter 3's last wq read (Tile WAR tracking orders this).
            wq_sb = wpool.tile([P, KT, QCOLS], BF16, name="wq_sb")
            wk_sb = wpool.tile([P, KT, HD], BF16, name="wk_sb")
            wv_sb = wpool.tile([P, KT, HD], BF16, name="wv_sb")
            cos_sb = wpool.tile([P, T], BF16, name="cos_sb")
            sin_sb = wpool.tile([P, T], BF16, name="sin_sb")
            if variant == "general":
                mk_sb = wpool.tile([P, STB, S], BF16, name="mk_sb")

            ident = cst.tile([P, P], BF16, name="ident")
            make_identity(nc, ident)
            ones_sq = cst.tile([P, P], BF16, name="ones_sq")
            nc.vector.memset(ones_sq[:], 1.0)
            # additive causal mask for a diagonal 128x128 span:
            # tri_neg[p, j] = 0 if p <= j else -1e9 (future keys killed
            # pre-exp, so no post-exp masking op is needed anywhere).
            zeros_sq = cst.tile([P, P], BF16, name="zeros_sq")
            nc.vector.memset(zeros_sq[:], 0.0)
            tri_neg = cst.tile([P, P], BF16, name="tri_neg")
            nc.gpsimd.affine_select(
                out=tri_neg[:], in_=zeros_sq[:],
                compare_op=mybir.AluOpType.is_ge, fill=-1e9,
                base=0, channel_multiplier=-1, pattern=[[1, P]])

            # ---- persistent activations ----
            q_rot = [[persist.tile([P, S], BF16, name=f"q{h}_{b}")
                      for b in range(B)] for h in range(HPC)]
            k_rot = [persist.tile([P, S], BF16, name=f"k{b}") for b in range(B)]
            v_sb = [persist.tile([P, STB, P], BF16, name=f"v{b}") for b in range(B)]
            attn = [[persist.tile([P, S], BF16, name=f"attn{h}_{b}")
                     for b in range(B)] for h in range(HPC)]

            # per-(batch, sq-half) gather buffers: two smaller collectives per
            # batch, each fired as soon as its attention chunk finishes --
            # spreads collective traffic and halves skew exposure.
            ag_in = [[dram.tile([HPC * P, QW], BF16, name=f"agin{b}_{c}")
                      for c in range(SQC)] for b in range(B)]
            ag_out = [[dram.tile([H * P, QW], BF16, addr_space="Shared",
                                 name=f"agout{b}_{c}") for c in range(SQC)]
                      for b in range(B)]

            def rope(dst, dst_off, src_bf, qoff):
                """RoPE on split layout (a=0:64, b=64:128), 4 DVE ops.
                p1 = [a*c; b*c];  p2sw = [b*(-s); a*s] computed directly with
                cross-partition reads (sin table already holds [s; -s]);
                dst = p1 + p2sw = [a*c - b*s; a*s + b*c]."""
                c = cos_sb[:, qoff:qoff + QW]
                p1 = rtp.tile([P, QW], BF16, name="p1")
                p2sw = rtp.tile([P, QW], BF16, name="p2sw")
                nc.vector.tensor_mul(p1[:], src_bf[:], c)
                nc.vector.tensor_mul(p2sw[0:64, :], src_bf[64:128, :],
                                     sin_sb[64:128, qoff:qoff + QW])
                nc.vector.tensor_mul(p2sw[64:128, :], src_bf[0:64, :],
                                     sin_sb[0:64, qoff:qoff + QW])
                nc.vector.tensor_add(dst[:, dst_off:dst_off + QW], p1[:], p2sw[:])

            def load_weights():
                """Stream projection weights + rope tables on the scalar
                (HWDGE) queue, ordered by first consumption under the
                quarter-0 schedule (K/V leading Q by QLAG k-tiles), so the
                sync queue only carries the xt stream and no weight arrives
                later than the matmul that needs it."""
                dma = nc.scalar.dma_start

                def tables(qx):
                    toff = qx * QW
                    dma(cos_sb[:, toff:toff + QW], cos_e[:, toff:toff + QW])
                    dma(sin_sb[:, toff:toff + QW], sin_e[:, toff:toff + QW])

                dma(wk_sb[:, 0:2, :], wk_e[:, 0:2, :])
                dma(wv_sb[:, 0:2, :], wv_e[:, 0:2, :])
                dma(wk_sb[:, 2:8, :], wk_e[:, 2:8, :])
                dma(wv_sb[:, 2:8, :], wv_e[:, 2:8, :])
                dma(wk_sb[:, 8:16, :], wk_e[:, 8:16, :])
                dma(wv_sb[:, 8:16, :], wv_e[:, 8:16, :])
                dma(wq_sb[:, 0:4, :], wq_e[:, 0:4, :])
                dma(wk_sb[:, 16:24, :], wk_e[:, 16:24, :])
                dma(wv_sb[:, 16:24, :], wv_e[:, 16:24, :])
                dma(wq_sb[:, 4:8, :], wq_e[:, 4:8, :])
                dma(wq_sb[:, 8:12, :], wq_e[:, 8:12, :])
                dma(wk_sb[:, 24:KT, :], wk_e[:, 24:KT, :])
                dma(wv_sb[:, 24:KT, :], wv_e[:, 24:KT, :])
                tables(0)
                dma(wq_sb[:, 12:16, :], wq_e[:, 12:16, :])
                dma(wq_sb[:, 16:20, :], wq_e[:, 16:20, :])
                tables(1)
                dma(wq_sb[:, 20:24, :], wq_e[:, 20:24, :])
                dma(wq_sb[:, 24:28, :], wq_e[:, 24:28, :])
                dma(wq_sb[:, 28:KT, :], wq_e[:, 28:KT, :])
                tables(2)
                tables(3)
                if variant == "general":
                    dma(mk_sb[:], mk_e[:])

            def wo_load():
                """Overwrite wq_sb with wo (WAR-ordered after the last wq read,
                i.e. streams during attention_batch(1)).  On the scalar queue so
                the sync queue only carries agt readback for wo_batch(0)."""
                for c in range(4):
                    nc.scalar.dma_start(wq_sb[:, 8 * c:8 * c + 8, :],
                                        wo_e[:, 8 * c:8 * c + 8, :])

            def proj_quarter(qx):
                b, boff = qx // 2, (qx % 2) * QW
                toff = qx * QW
                # psums: kv pair (k | v), q0, q1 single banks, q23 pair.
                kv_ps = ps2.tile([P, 2 * QW], FP32, name="kv_ps", tag="pair")
                q01 = [ps1.tile([P, QW], FP32, name="q01", tag="ps")
                       for _ in range(2)]
                q23_ps = ps2.tile([P, 2 * QW], FP32, name="q23_ps", tag="pair")

                def mm(mb, k, xt, start, stop):
                    if mb == 0:
                        w, dst = wk_sb[:, k, :], kv_ps[:, 0:QW]
                    elif mb == 1:
                        w, dst = wv_sb[:, k, :], kv_ps[:, QW:2 * QW]
                    elif mb < 4:
                        w = wq_sb[:, k, (mb - 2) * P:(mb - 1) * P]
                        dst = q01[mb - 2][:]
                    else:
                        w = wq_sb[:, k, (mb - 2) * P:(mb - 1) * P]
                        dst = q23_ps[:, (mb - 4) * QW:(mb - 3) * QW]
                    nc.tensor.matmul(dst, w, xt, start=start, stop=stop)

                def kvmm(k, xt):
                    mm(0, k, xt, k == 0, k == KT - 1)
                    mm(1, k, xt, k == 0, k == KT - 1)

                def qmm(k, xt, mbs=(2, 3, 4, 5)):
                    for mb in mbs:
                        mm(mb, k, xt, k == 0, k == KT - 1)

                evicted = []

                def evict_kv():
                    ke = evp.tile([P, QW], BF16, name="ke", tag="qe")
                    nc.scalar.copy(ke[:], kv_ps[:, 0:QW])
                    rope(k_rot[b], boff, ke, toff)
                    ve = evp.tile([P, QW], BF16, name="ve", tag="qe")
                    nc.scalar.copy(ve[:], kv_ps[:, QW:2 * QW])
                    evicted.append(ve)

                def evict_q(h):
                    qe = evp.tile([P, QW], BF16, name="qe", tag="qe")
                    if h < 2:
                        nc.scalar.copy(qe[:], q01[h][:])
                    else:
                        nc.scalar.copy(qe[:], q23_ps[:, (h - 2) * QW:(h - 1) * QW])
                    rope(q_rot[h][b], boff, qe, toff)

                tps = []

                def v_transposes():
                    ve = evicted[0]
                    for i in range(QW // P):
                        tp = ps1.tile([P, P], BF16, name="tp", tag="ps")
                        nc.tensor.transpose(tp[:], ve[:, i * P:(i + 1) * P],
                                            ident[:])
                        tps.append(tp)

                def v_copies(eng=None):
                    for i, tp in enumerate(tps):
                        st = (qx % 2) * 4 + i
                        if eng == "vector":
                            nc.vector.tensor_copy(v_sb[b][:, st, :], tp[:])
                        else:
                            nc.scalar.copy(v_sb[b][:, st, :], tp[:])

                xts = {}
                if qx == 0:
                    # K/V lead Q by QLAG k-tiles so wq k-tiles have an extra
                    # ~12us of HBM-stream slack during the cold start.
                    chunks = [1, 1, 2, 2, 2] + [XTC] * 6
                    k0 = 0
                    qptr = 0
                    for cw in chunks:
                        xt4 = xtp.tile([P, XTC, QW], BF16, name="xt4")
                        nc.sync.dma_start(xt4[:, 0:cw, :],
                                          xt_e[:, qx, k0:k0 + cw, :])
                        for j in range(cw):
                            xts[k0 + j] = xt4[:, j, :]
                            kvmm(k0 + j, xts[k0 + j])
                        k0 += cw
                        # cap the catch-up so a 12-ktile Q tail remains: its
                        # mb-major groups are long enough that each head's
                        # eviction + rope completes during the next group.
                        while qptr <= min(k0 - 1 - QLAG, KT - 13):
                            qmm(qptr, xts.pop(qptr))
                            qptr += 1
                    evict_kv()
                    # Q tail mb-major: early heads stop first so their
                    # evictions + ropes overlap the remaining matmuls.
                    for mb in (2, 3, 4, 5):
                        for kq in range(qptr, KT):
                            mm(mb, kq, xts[kq], kq == 0, kq == KT - 1)
                        evict_q(mb - 2)
                        if mb == 3:
                            v_transposes()
                    v_copies()
                else:
                    chunks = [XTC] * (KT // XTC)
                    k0 = 0
                    tail = []
                    for ci, cw in enumerate(chunks):
                        xt4 = xtp.tile([P, XTC, QW], BF16, name="xt4")
                        nc.sync.dma_start(xt4[:, 0:cw, :],
                                          xt_e[:, qx, k0:k0 + cw, :])
                        if ci < len(chunks) - 3:
                            for j in range(cw):
                                k = k0 + j
                                kvmm(k, xt4[:, j, :])
                                qmm(k, xt4[:, j, :])
                        else:
                            tail += [(k0 + j, xt4[:, j, :]) for j in range(cw)]
                        k0 += cw
                    # 12-ktile tail, mb-major ordered K, q0..q3, V: each
                    # eviction + rope (a ~2.7us scalar+DVE chain) completes
                    # during the following ~2.8us matmul group, so every rope
                    # the next attention phase needs is ready when its first
                    # QK issues.  V runs last: its eviction chain (transposes
                    # + v_sb copies on DVE) only gates the PV matmuls ~2.5us
                    # into the attention phase.
                    for k, xt in tail:
                        mm(0, k, xt, k == 0, k == KT - 1)
                    ke = evp.tile([P, QW], BF16, name="ke", tag="qe")
                    nc.scalar.copy(ke[:], kv_ps[:, 0:QW])
                    rope(k_rot[b], boff, ke, toff)
                    for mb in (2, 3, 4, 5):
                        for k, xt in tail:
                            mm(mb, k, xt, k == 0, k == KT - 1)
                        evict_q(mb - 2)
                    for k, xt in tail:
                        mm(1, k, xt, k == 0, k == KT - 1)
                    ve = evp.tile([P, QW], BF16, name="ve", tag="qe")
                    nc.scalar.copy(ve[:], kv_ps[:, QW:2 * QW])
                    evicted.append(ve)
                    v_transposes()
                    v_copies(eng="vector")

            def attention_batch(b, sqcs=tuple(range(SQC))):
                if variant == "general":
                    attention_batch_general(b, sqcs)
                    return
                for sqc in sqcs:
                    sq0 = sqc * QW
                    stl = _st_info(variant, sqc)
                    # interleave full-width and narrowed st blocks so the
                    # scalar engine's exp stream (cost ~ width) never falls
                    # behind the PE during a run of full-width blocks; keep a
                    # full-width block first (the dacc copy needs full width).
                    full = [e for e in stl if e[1] == 0]
                    narrow = sorted([e for e in stl if e[1] > 0],
                                    key=lambda e: -e[1])
                    stl = []
                    while full or narrow:
                        if full:
                            stl.append(full.pop(0))
                        if narrow:
                            stl.append(narrow.pop())
                    n = len(stl)
                    pairs = ((0, 1), (2, 3))
                    pso = [ps1.tile([P, QW], FP32, name="pso", tag="ps")
                           for _ in range(HPC)]
                    dacc = [dap.tile([P, QW], BF16, name="dacc")
                            for _ in range(HPC)]
                    prs = []

                    def pv(i):
                        st, r, sel = stl[i]
                        for pi, hh in enumerate(pairs):
                            pr = prs[i][pi]
                            for sl, h in enumerate(hh):
                                src = pr[:, r:QW] if sl == 0 \
                                    else pr[:, QW:2 * QW - r]
                                nc.tensor.matmul(pso[h][:, r:QW],
                                                 v_sb[b][:, st, :], src,
                                                 start=(i == 0),
                                                 stop=(i == n - 1))

                    def finish_head(h):
                        # denominator broadcast + normalize + ship to gather
                        psd = ps2.tile([P, 2 * QW], FP32, name="psd",
                                       tag="pair")
                        nc.tensor.matmul(psd[:, 0:QW], ones_sq[:], dacc[h][:],
                                         start=True, stop=True)
                        rb = mip.tile([P, QW], FP32, name="rb")
                        nc.vector.reciprocal_approx_fast(rb[:], psd[:, 0:QW])
                        nc.vector.tensor_mul(attn[h][b][:, sq0:sq0 + QW],
                                             pso[h][:], rb[:])
                        nc.gpsimd.dma_start(
                            ag_in[b][sqc][h * P:(h + 1) * P, :],
                            attn[h][b][:, sq0:sq0 + QW])

                    prs.extend([None, None] for _ in range(n))

                    def emit_pair(i, pi):
                        st, r, sel = stl[i]
                        assert sel is None or sel == r
                        hh = pairs[pi]
                        # both heads of the pair in one 2-bank psum; the
                        # causal-narrowed spans pack adjacently so one exp
                        # covers both heads with no wasted columns:
                        # h0 at [r:QW], h1 at [QW:2QW-r].
                        pss = ps2.tile([P, 2 * QW], FP32, name="pss",
                                       tag="pair")
                        kst = k_rot[b][:, st * P:(st + 1) * P]
                        if sel is not None:
                            # additive causal mask preloaded into the
                            # diagonal span of each head's score bank:
                            # exp then yields exact zeros for future
                            # keys, keeping the PV chain PE+ACT-only.
                            nc.tensor.matmul(pss[:, r:r + P], ident[:],
                                             tri_neg[:],
                                             start=True, stop=False)
                            nc.tensor.matmul(pss[:, QW:QW + P], ident[:],
                                             tri_neg[:],
                                             start=True, stop=False)
                        nc.tensor.matmul(
                            pss[:, r:QW], kst,
                            q_rot[hh[0]][b][:, sq0 + r:sq0 + QW],
                            start=(sel is None), stop=True)
                        nc.tensor.matmul(
                            pss[:, QW:2 * QW - r], kst,
                            q_rot[hh[1]][b][:, sq0 + r:sq0 + QW],
                            start=(sel is None), stop=True)
                        pr = prp.tile([P, 2 * QW], BF16, name="pr",
                                      tag="pr")
                        if i == 0:
                            # per-head exps at the chunk's first block: each
                            # starts one QK earlier and is half as long, so
                            # the second block's QKs (which reuse this psum
                            # slot) stall ~1us less while the exp pipeline
                            # fills.
                            nc.scalar.activation(pr[:, r:QW],
                                                 pss[:, r:QW], EXP)
                            nc.scalar.activation(pr[:, QW:2 * QW - r],
                                                 pss[:, QW:2 * QW - r], EXP)
                        else:
                            nc.scalar.activation(pr[:, r:2 * QW - r],
                                                 pss[:, r:2 * QW - r], EXP)
                        # denominator partial sums on DVE (bf16); only
                        # consumed by the ones-matmul at chunk end, so
                        # DVE lag never stalls the PE.  The last two blocks'
                        # adds are deferred and interleaved per-head with
                        # finish_head so recip(h0) (whose completion
                        # releases the next proj phase's PSUM banks via
                        # WAR) runs as early as possible.
                        if i < n - 2:
                            for sl, h in enumerate(hh):
                                src = pr[:, r:QW] if sl == 0 \
                                    else pr[:, QW:2 * QW - r]
                                if i == 0:
                                    nc.vector.tensor_copy(dacc[h][:], src)
                                else:
                                    nc.vector.tensor_add(
                                        dacc[h][:, r:QW],
                                        dacc[h][:, r:QW], src)
                        prs[i][pi] = pr

                    for i in range(n):
                        emit_pair(i, 0)
                        emit_pair(i, 1)
                        if i >= 1:
                            pv(i - 1)
                    pv(n - 1)
                    for h in range(HPC):
                        for i in (n - 2, n - 1):
                            st, r, sel = stl[i]
                            pr = prs[i][h // 2]
                            src = pr[:, r:QW] if h % 2 == 0 \
                                else pr[:, QW:2 * QW - r]
                            nc.vector.tensor_add(dacc[h][:, r:QW],
                                                 dacc[h][:, r:QW], src)
                        finish_head(h)

            def attention_batch_general(b, sqcs):
                for sqc in sqcs:
                    sq0 = sqc * QW
                    stl = _st_info(variant, sqc)
                    n = len(stl)
                    for h in range(HPC):
                        prtiles = []
                        for st, r, sel in stl:
                            pss = ps1.tile([P, QW], FP32, name="pss", tag="ps")
                            nc.tensor.matmul(pss[:], ident[:],
                                             mk_sb[:, st, sq0:sq0 + QW],
                                             start=True, stop=False)
                            nc.tensor.matmul(
                                pss[:], k_rot[b][:, st * P:(st + 1) * P],
                                q_rot[h][b][:, sq0:sq0 + QW],
                                start=False, stop=True)
                            pr = prp.tile([P, QW], BF16, name="pr", tag="pr")
                            nc.scalar.activation(pr[:], pss[:], EXP)
                            prtiles.append(pr)
                        pso = ps2.tile([P, 2 * QW], FP32, name="psog",
                                       tag="pair")
                        for i, (st, r, sel) in enumerate(stl):
                            pr = prtiles[i]
                            nc.tensor.matmul(pso[:, 0:QW], v_sb[b][:, st, :],
                                             pr[:],
                                             start=(i == 0), stop=(i == n - 1))
                            nc.tensor.matmul(pso[:, QW:2 * QW], ones_sq[:],
                                             pr[:],
                                             start=(i == 0), stop=(i == n - 1))
                        rb = mip.tile([P, QW], FP32, name="rb")
                        nc.vector.reciprocal_approx_fast(rb[:],
                                                         pso[:, QW:2 * QW])
                        nc.vector.tensor_mul(attn[h][b][:, sq0:sq0 + QW],
                                             pso[:, 0:QW], rb[:])
                        nc.gpsimd.dma_start(
                            ag_in[b][sqc][h * P:(h + 1) * P, :],
                            attn[h][b][:, sq0:sq0 + QW])

            def gather_batch(b, c):
                nc.gpsimd.collective_compute(
                    "AllGather", mybir.AluOpType.bypass,
                    ins=[ag_in[b][c][:].opt()],
                    outs=[ag_out[b][c][:].opt()],
                    replica_groups=[list(range(NCORES))],
                )

            def wo_chunk(b, nch, last):
                """wo matmuls for one (batch, sq-chunk): full 4096-contraction
                over 512 tokens.  Processing sq-chunks serially (not
                interleaved) means chunk 0's matmuls only need that chunk's
                AllGather -- chunk 1's gather (the last collective for the
                batch) gets an extra ~35us of slack before first use."""
                ag_r = ag_out[b][nch].rearrange("(k p) t -> p k t", p=P)
                psw_pair = ps2.tile([P, 2 * QW], FP32, name="psw_pair",
                                    tag="pair")
                psw_s = [ps1.tile([P, QW], FP32, name="psw", tag="ps")
                         for _ in range(2)]

                def psw(mb):
                    if mb < 2:
                        return psw_pair[:, mb * QW:(mb + 1) * QW]
                    return psw_s[mb - 2][:]

                nchk = KT // AGC
                wtail = []
                for kc in range(nchk):
                    agt = agp.tile([P, AGC, QW], BF16, name="agt")
                    nc.sync.dma_start(agt[:],
                                      ag_r[:, kc * AGC:(kc + 1) * AGC, :])
                    if kc < nchk - 2:
                        for j in range(AGC):
                            k = kc * AGC + j
                            for mb in range(4):
                                w = wq_sb[:, k, mb * P:(mb + 1) * P]
                                nc.tensor.matmul(
                                    psw(mb), w, agt[:, j, :],
                                    start=(k == 0), stop=False)
                    else:
                        wtail += [(kc * AGC + j, agt) for j in range(AGC)]
                # last two chunks mb-major so early mb groups stop several us
                # before the end and their evictions + out DMAs overlap the
                # remaining matmuls.
                tcol = b * S + nch * QW
                for mb in range(4):
                    for k, agt in wtail:
                        w = wq_sb[:, k, mb * P:(mb + 1) * P]
                        nc.tensor.matmul(
                            psw(mb), w, agt[:, k % AGC, :],
                            start=False, stop=(k == KT - 1))
                    if mb == 3 and last:
                        # split the very last eviction in half across
                        # engines/queues so its copy + DMA pipeline
                        # instead of serializing after the final matmul.
                        hw = QW // 2
                        for hf in range(2):
                            ow = owp.tile([P, hw], BF16, name="owh",
                                          tag="owh")
                            src = psw(mb)[:, hf * hw:(hf + 1) * hw]
                            if hf == 0:
                                nc.vector.tensor_copy(ow[:], src)
                                dma = nc.sync.dma_start
                            else:
                                nc.scalar.copy(ow[:], src)
                                dma = nc.scalar.dma_start
                            dma(out_e[mb * P:(mb + 1) * P,
                                      tcol + hf * hw:tcol + (hf + 1) * hw],
                                ow[:])
                        continue
                    ow = owp.tile([P, QW], BF16, name="ow")
                    if mb % 2 == 0:
                        nc.scalar.copy(ow[:], psw(mb))
                        dma = nc.scalar.dma_start
                    else:
                        nc.vector.tensor_copy(ow[:], psw(mb))
                        dma = nc.sync.dma_start
                    dma(out_e[mb * P:(mb + 1) * P, tcol:tcol + QW], ow[:])

            def wo_batch(b):
                for nch in range(SQC):
                    wo_chunk(b, nch, last=(b == B - 1 and nch == SQC - 1))

            # ---- timeline ----
            # causal: attention sq-chunks interleave between proj quarters --
            # chunk s0 of batch b only needs that batch's first token quarter
            # (no future keys).  This fires gather(0) earlier and gives the
            # xt/weight streams HBM-quiet windows (attention phases do no HBM
            # traffic) to get ahead.  Non-causal variants attend future keys,
            # so each batch's attention must wait for BOTH its quarters.
            load_weights()
            if variant == "causal":
                proj_quarter(0)
                attention_batch(0, (0,))
                gather_batch(0, 0)
                proj_quarter(1)
                attention_batch(0, (1,))
                gather_batch(0, 1)
                proj_quarter(2)
                attention_batch(1, (0,))
                gather_batch(1, 0)
                proj_quarter(3)
                wo_load()
                attention_batch(1, (1,))
                gather_batch(1, 1)
            else:
                proj_quarter(0)
                proj_quarter(1)
                attention_batch(0, (0,))
                gather_batch(0, 0)
                attention_batch(0, (1,))
                gather_batch(0, 1)
                proj_quarter(2)
                proj_quarter(3)
                wo_load()
                attention_batch(1, (0,))
                gather_batch(1, 0)
                attention_batch(1, (1,))
                gather_batch(1, 1)
            wo_batch(0)
            wo_batch(1)

    nc.compile()
    return nc


def _get_compiled(variant):
    if variant not in _COMPILED:
        _COMPILED[variant] = _build(variant)
    return _COMPILED[variant]


def _detect_variant(mask2d):
    if not np.any(mask2d):
        return "nomask"
    tril = np.tril(mask2d)
    if not np.any(tril):
        iu = np.triu_indices(S, 1)
        if np.all(mask2d[iu] <= -1e8):
            return "causal"
    return "general"


def _pack_kt(w):
    """[R*128, N] -> [128, R, N] so that [:, k, :] is rows k*128..k*128+127."""
    return np.ascontiguousarray(w.reshape(w.shape[0] // P, P, -1).transpose(1, 0, 2))


def kernel(x, wq, wk, wv, wo, lora_q_a, lora_q_b, lora_v_a, lora_v_b,
           freqs_cos, freqs_sin, mask, start_pos=0, **_):
    global LAST_RESULTS
    bf = ml_dtypes.bfloat16
    x = np.asarray(x, np.float32)
    wq = np.asarray(wq, np.float32)
    wk = np.asarray(wk, np.float32)
    wv = np.asarray(wv, np.float32)
    wo = np.asarray(wo, np.float32)
    lora_q_a = np.asarray(lora_q_a, np.float32)
    lora_q_b = np.asarray(lora_q_b, np.float32)
    lora_v_a = np.asarray(lora_v_a, np.float32)
    lora_v_b = np.asarray(lora_v_b, np.float32)
    cos = np.asarray(freqs_cos, np.float32)
    sin = np.asarray(freqs_sin, np.float32)
    mask2d = np.asarray(mask, np.float32).reshape(S, S)

    variant = _detect_variant(mask2d)
    nc = _get_compiled(variant)

    # fold LoRA + scale; permute rope pairs (evens then odds within each head)
    wq_eff = (wq + lora_q_a @ lora_q_b) * np.float32(1.0 / np.sqrt(HD))
    wv_eff = wv + lora_v_a @ lora_v_b
    perm = np.concatenate([np.arange(0, HD, 2), np.arange(1, HD, 2)])
    qperm = (np.arange(H)[:, None] * HD + perm[None, :]).reshape(-1)
    kperm = (np.arange(KVH)[:, None] * HD + perm[None, :]).reshape(-1)
    wq_eff = wq_eff[:, qperm]
    wk_p = wk[:, kperm]

    xt = np.ascontiguousarray(x.reshape(T, D).T)        # [4096, 2048]
    # [128, KT, T] -> quarter-major [128, NQ, KT, QW] (contiguous per chunk)
    xt_p = np.ascontiguousarray(
        _pack_kt(xt).reshape(P, KT, NQ, QW).transpose(0, 2, 1, 3)).astype(bf)
    c64 = np.tile(cos.T, (1, B))                        # [64, 2048]
    s64 = np.tile(sin.T, (1, B))
    cosT = np.concatenate([c64, c64], axis=0).astype(bf)   # [c; c]
    sinT = np.concatenate([s64, -s64], axis=0).astype(bf)  # [s; -s]

    if variant == "general":
        maskT = np.ascontiguousarray(mask2d.T)          # [st, sq]
        mk = _pack_kt(maskT).astype(bf)                 # [128, 8, 1024]
    else:
        mk = None

    in_maps = []
    for c in range(NCORES):
        im = {
            "xt": xt_p,
            "wq": _pack_kt(wq_eff[:, c * QCOLS:(c + 1) * QCOLS]).astype(bf),
            "wk": _pack_kt(wk_p[:, c * HD:(c + 1) * HD]).astype(bf),
            "wv": _pack_kt(wv_eff[:, c * HD:(c + 1) * HD]).astype(bf),
            "wo": _pack_kt(wo[:, c * QCOLS:(c + 1) * QCOLS]).astype(bf),
            "cos": cosT,
            "sin": sinT,
        }
        if mk is not None:
            im["mk"] = mk
        in_maps.append(im)

    res = run_bass_kernel_spmd(nc, in_maps, core_ids=list(range(NCORES)))
    LAST_RESULTS = res
    outT = np.concatenate([res.results[c]["out"] for c in range(NCORES)], axis=0)
    return np.ascontiguousarray(outT.T).reshape(B, S, D).astype(np.float32)


# revision 37
# speedup vs baseline: 1.0156x; 1.0000x over previous
"""Distributed Trainium2 Bass kernel for nn_Attention (GQA attention + LoRA + RoPE).

Sharding: tensor-parallel over heads across 8 NeuronCores.
  - core c owns Q heads 4c..4c+3 and KV head c (GQA group).
  - wq/wk/wv column-sharded; wo COLUMN-sharded (each core computes a
    512-column slice of the output over the full 4096 contraction, fed by an
    AllGather of all cores' per-head attention outputs).
  - LoRA is folded into wq/wv on the host (x@wq + (x@A)@B == x@(wq + A@B)).
  - 1/sqrt(HD) folded into wq.
  - RoPE pair permutation folded into wq/wk column order: within each head the
    even dims come first, odd dims second, so on-device RoPE is plain
    elementwise math on partition halves.

Everything the device computes is bf16-in/f32-accumulate.

v3 performance notes (vs the 432us v2):
  The chip runs GPIO-power-throttled to 13/16 (~1.95GHz PE) for ~90% of the
  kernel, so v3 attacks streamed-column count and non-PE stalls:
  - softmax denominator no longer uses per-st ones-matmuls: pr tiles are
    summed on DVE (bf16, lag-tolerant) and a single ones-matmul per
    (head, chunk) broadcasts it (-28.7K PE columns).  The last two blocks'
    adds interleave per-head with the finish chain so the next phase's
    PSUM WARs release early.
  - QK head-pairs pack into one 2-bank PSUM tile with the causal-narrowed
    spans adjacent, so ONE exp ACTIVATE covers two heads with no wasted
    columns (scalar exp time 58us -> 22us); the first block uses per-head
    exps to prime the pipeline.
  - causal masking preloaded as a -1e9 lower-tri matmul into the diagonal
    span of the score psum (exp yields exact zeros): the QK->exp->PV chain
    touches only PE+ACT, never DVE/gpsimd.
  - PV matmuls lag QK by one st block so exp latency never stalls the PE;
    full/narrow st blocks interleave so exp cost tracks PE cost.
  - proj quarter 0 runs K/V matmuls 8 k-tiles ahead of Q matmuls with the
    weight stream ordered to match (kills a 7.5us HBM-starvation stall);
    quarters 1-3 defer a 12-ktile tail ordered K,q0..q3,V so every rope
    the next attention phase needs completes during the tail (4-op ropes,
    v_sb copies on DVE).
  - wo processes each (batch, sq-chunk) serially over the full contraction,
    so the last AllGather gets ~35us of slack before first use (collective
    skew no longer stalls the PE); last eviction split across engines.
  - PSUM: 2-bank pair tiles (kv / q23 in proj, QK pairs in attention,
    2 of 4 wo accumulators) + 4 single banks, exactly filling 8 banks.
  - non-causal masks attend future keys, so those variants run each batch's
    attention only after BOTH its token quarters are projected (the v2
    schedule read uninitialized k/v for them).
"""

import sys
import types

import numpy as np
import ml_dtypes

import concourse.bass as bass
from concourse import bacc
import concourse.mybir as mybir
import concourse.tile as tile
from concourse.bass_utils import run_bass_kernel_spmd
from concourse.masks import make_identity


def _ensure_axon_hooks():
    """run_bass_kernel_spmd(trace=True) imports antenv.axon_hooks, which some
    images lack; install a no-op shim so a BASS_TRACE env var can't crash us."""
    try:
        import antenv
    except ImportError:
        return
    if "antenv.axon_hooks" in sys.modules:
        return
    try:
        from antenv import axon_hooks  # noqa: F401
        return
    except ImportError:
        pass
    mod = types.ModuleType("antenv.axon_hooks")
    mod._hook = None
    mod.set_axon_ntff_profile_hook = lambda h: setattr(mod, "_hook", h)
    mod.get_axon_ntff_profile_hook = lambda: mod._hook
    sys.modules["antenv.axon_hooks"] = mod
    antenv.axon_hooks = mod


_ensure_axon_hooks()

B, S, D = 2, 1024, 4096
H, KVH, HD = 32, 8, 128
NCORES = 8
HPC = H // NCORES            # 4 q heads per core
QCOLS = HPC * HD             # 512
T = B * S                    # 2048
P = 128
KT = D // P                  # 32 k tiles
NQ = 4                       # token quarters (512 tokens each)
QW = T // NQ                 # 512
SQC = 2                      # sq chunks per batch
STB = S // P                 # 8 st blocks per batch
XTC = 4                      # k-tiles per xt DMA chunk
AGC = 2                      # k-tiles per allgather-readback DMA chunk
QLAG = 8                     # quarter-0 K/V lead over Q, in k-tiles

FP32 = mybir.dt.float32
BF16 = mybir.dt.bfloat16
EXP = mybir.ActivationFunctionType.Exp

_COMPILED = {}
LAST_RESULTS = None


def _st_info(variant, sqc):
    """st blocks contributing to sq chunk sqc, as (st, r, sel):
    r = first needed column within the 512-wide chunk (0 for full width),
    sel = start of the 128-wide diagonal span needing triangular zeroing
    (None if the block is fully below the diagonal / no mask)."""
    out = []
    for st in range(STB):
        if variant == "causal":
            rd = st * P - sqc * QW
            if rd >= QW:
                continue  # fully masked
            if rd >= 0:
                out.append((st, rd, rd))
            else:
                out.append((st, 0, None))
        else:
            out.append((st, 0, None))
    return out


def _build(variant):
    nc = bacc.Bacc(None)

    # xt packed quarter-major: [:, qx, k, :] is per-partition contiguous 4KB
    # per 4-ktile chunk, so xt chunk DMAs run at large-descriptor efficiency.
    xt_e = nc.declare_dram_parameter("xt", [P, NQ, KT, QW], BF16, isOutput=False)
    wq_e = nc.declare_dram_parameter("wq", [P, KT, QCOLS], BF16, isOutput=False)
    wk_e = nc.declare_dram_parameter("wk", [P, KT, HD], BF16, isOutput=False)
    wv_e = nc.declare_dram_parameter("wv", [P, KT, HD], BF16, isOutput=False)
    wo_e = nc.declare_dram_parameter("wo", [P, KT, QCOLS], BF16, isOutput=False)
    # cos: [c; c] duplicated halves.  sin: [s; -s] (negated bottom half).
    cos_e = nc.declare_dram_parameter("cos", [P, T], BF16, isOutput=False)
    sin_e = nc.declare_dram_parameter("sin", [P, T], BF16, isOutput=False)
    if variant == "general":
        mk_e = nc.declare_dram_parameter("mk", [P, STB, S], BF16, isOutput=False)
    out_e = nc.declare_dram_parameter("out", [QCOLS, T], BF16, isOutput=True)

    with tile.TileContext(nc) as tc:
        with (
            tc.tile_pool(name="wpool", bufs=1) as wpool,
            tc.tile_pool(name="cst", bufs=1) as cst,
            tc.tile_pool(name="persist", bufs=1) as persist,
            tc.tile_pool(name="xt", bufs=10 if variant != "general" else 8) as xtp,
            tc.tile_pool(name="ev", bufs=4) as evp,
            tc.tile_pool(name="rt", bufs=3) as rtp,
            tc.tile_pool(name="probs", bufs=7 if variant != "general" else 20) as prp,
            tc.tile_pool(name="dacc", bufs=4) as dap,
            tc.tile_pool(name="misc", bufs=3) as mip,
            tc.tile_pool(name="ag", bufs=12 if variant != "general" else 6) as agp,
            tc.tile_pool(name="ow", bufs=8) as owp,
            tc.tile_pool(name="ps2", bufs=2, space="PSUM") as ps2,
            tc.tile_pool(name="ps1", bufs=4, space="PSUM") as ps1,
            tc.tile_pool(name="dram", bufs=1, space="DRAM") as dram,
        ):
            # ---- resident weights / constants ----
            # wq_sb doubles as wo storage: wo is DMA'd over it after proj
            # quarter 3's last wq read (Tile WAR tracking orders this).
            wq_sb = wpool.tile([P, KT, QCOLS], BF16, name="wq_sb")
            wk_sb = wpool.tile([P, KT, HD], BF16, name="wk_sb")
            wv_sb = wpool.tile([P, KT, HD], BF16, name="wv_sb")
            cos_sb = wpool.tile([P, T], BF16, name="cos_sb")
            sin_sb = wpool.tile([P, T], BF16, name="sin_sb")
            if variant == "general":
                mk_sb = wpool.tile([P, STB, S], BF16, name="mk_sb")

            ident = cst.tile([P, P], BF16, name="ident")
            make_identity(nc, ident)
            ones_sq = cst.tile([P, P], BF16, name="ones_sq")
            nc.vector.memset(ones_sq[:], 1.0)
            # additive causal mask for a diagonal 128x128 span:
            # tri_neg[p, j] = 0 if p <= j else -1e9 (future keys killed
            # pre-exp, so no post-exp masking op is needed anywhere).
            zeros_sq = cst.tile([P, P], BF16, name="zeros_sq")
            nc.vector.memset(zeros_sq[:], 0.0)
            tri_neg = cst.tile([P, P], BF16, name="tri_neg")
            nc.gpsimd.affine_select(
                out=tri_neg[:], in_=zeros_sq[:],
                compare_op=mybir.AluOpType.is_ge, fill=-1e9,
                base=0, channel_multiplier=-1, pattern=[[1, P]])

            # ---- persistent activations ----
            q_rot = [[persist.tile([P, S], BF16, name=f"q{h}_{b}")
                      for b in range(B)] for h in range(HPC)]
            k_rot = [persist.tile([P, S], BF16, name=f"k{b}") for b in range(B)]
            v_sb = [persist.tile([P, STB, P], BF16, name=f"v{b}") for b in range(B)]
            attn = [[persist.tile([P, S], BF16, name=f"attn{h}_{b}")
                     for b in range(B)] for h in range(HPC)]

            # per-(batch, sq-half) gather buffers: two smaller collectives per
            # batch, each fired as soon as its attention chunk finishes --
            # spreads collective traffic and halves skew exposure.
            ag_in = [[dram.tile([HPC * P, QW], BF16, name=f"agin{b}_{c}")
                      for c in range(SQC)] for b in range(B)]
            ag_out = [[dram.tile([H * P, QW], BF16, addr_space="Shared",
                                 name=f"agout{b}_{c}") for c in range(SQC)]
                      for b in range(B)]

            def rope(dst, dst_off, src_bf, qoff):
                """RoPE on split layout (a=0:64, b=64:128), 4 DVE ops.
                p1 = [a*c; b*c];  p2sw = [b*(-s); a*s] computed directly with
                cross-partition reads (sin table already holds [s; -s]);
                dst = p1 + p2sw = [a*c - b*s; a*s + b*c]."""
                c = cos_sb[:, qoff:qoff + QW]
                p1 = rtp.tile([P, QW], BF16, name="p1")
                p2sw = rtp.tile([P, QW], BF16, name="p2sw")
                nc.vector.tensor_mul(p1[:], src_bf[:], c)
                nc.vector.tensor_mul(p2sw[0:64, :], src_bf[64:128, :],
                                     sin_sb[64:128, qoff:qoff + QW])
                nc.vector.tensor_mul(p2sw[64:128, :], src_bf[0:64, :],
                                     sin_sb[0:64, qoff:qoff + QW])
                nc.vector.tensor_add(dst[:, dst_off:dst_off + QW], p1[:], p2sw[:])

            def load_weights():
                """Stream projection weights + rope tables on the scalar
                (HWDGE) queue, ordered by first consumption under the
                quarter-0 schedule (K/V leading Q by QLAG k-tiles), so the
                sync queue only carries the xt stream and no weight arrives
                later than the matmul that needs it."""
                dma = nc.scalar.dma_start

                def tables(qx):
                    toff = qx * QW
                    dma(cos_sb[:, toff:toff + QW], cos_e[:, toff:toff + QW])
                    dma(sin_sb[:, toff:toff + QW], sin_e[:, toff:toff + QW])

                dma(wk_sb[:, 0:2, :], wk_e[:, 0:2, :])
                dma(wv_sb[:, 0:2, :], wv_e[:, 0:2, :])
                dma(wk_sb[:, 2:8, :], wk_e[:, 2:8, :])
                dma(wv_sb[:, 2:8, :], wv_e[:, 2:8, :])
                dma(wk_sb[:, 8:16, :], wk_e[:, 8:16, :])
                dma(wv_sb[:, 8:16, :], wv_e[:, 8:16, :])
                dma(wq_sb[:, 0:4, :], wq_e[:, 0:4, :])
                dma(wk_sb[:, 16:24, :], wk_e[:, 16:24, :])
                dma(wv_sb[:, 16:24, :], wv_e[:, 16:24, :])
                dma(wq_sb[:, 4:8, :], wq_e[:, 4:8, :])
                dma(wq_sb[:, 8:12, :], wq_e[:, 8:12, :])
                dma(wk_sb[:, 24:KT, :], wk_e[:, 24:KT, :])
                dma(wv_sb[:, 24:KT, :], wv_e[:, 24:KT, :])
                tables(0)
                dma(wq_sb[:, 12:16, :], wq_e[:, 12:16, :])
                dma(wq_sb[:, 16:20, :], wq_e[:, 16:20, :])
                tables(1)
                dma(wq_sb[:, 20:24, :], wq_e[:, 20:24, :])
                dma(wq_sb[:, 24:28, :], wq_e[:, 24:28, :])
                dma(wq_sb[:, 28:KT, :], wq_e[:, 28:KT, :])
                tables(2)
                tables(3)
                if variant == "general":
                    dma(mk_sb[:], mk_e[:])

            def wo_load():
                """Overwrite wq_sb with wo (WAR-ordered after the last wq read,
                i.e. streams during attention_batch(1)).  On the scalar queue so
                the sync queue only carries agt readback for wo_batch(0)."""
                for c in range(4):
                    nc.scalar.dma_start(wq_sb[:, 8 * c:8 * c + 8, :],
                                        wo_e[:, 8 * c:8 * c + 8, :])

            def proj_quarter(qx):
                b, boff = qx // 2, (qx % 2) * QW
                toff = qx * QW
                # psums: kv pair (k | v), q0, q1 single banks, q23 pair.
                kv_ps = ps2.tile([P, 2 * QW], FP32, name="kv_ps", tag="pair")
                q01 = [ps1.tile([P, QW], FP32, name="q01", tag="ps")
                       for _ in range(2)]
                q23_ps = ps2.tile([P, 2 * QW], FP32, name="q23_ps", tag="pair")

                def mm(mb, k, xt, start, stop):
                    if mb == 0:
                        w, dst = wk_sb[:, k, :], kv_ps[:, 0:QW]
                    elif mb == 1:
                        w, dst = wv_sb[:, k, :], kv_ps[:, QW:2 * QW]
                    elif mb < 4:
                        w = wq_sb[:, k, (mb - 2) * P:(mb - 1) * P]
                        dst = q01[mb - 2][:]
                    else:
                        w = wq_sb[:, k, (mb - 2) * P:(mb - 1) * P]
                        dst = q23_ps[:, (mb - 4) * QW:(mb - 3) * QW]
                    nc.tensor.matmul(dst, w, xt, start=start, stop=stop)

                def kvmm(k, xt):
                    mm(0, k, xt, k == 0, k == KT - 1)
                    mm(1, k, xt, k == 0, k == KT - 1)

                def qmm(k, xt, mbs=(2, 3, 4, 5)):
                    for mb in mbs:
                        mm(mb, k, xt, k == 0, k == KT - 1)

                evicted = []

                def evict_kv():
                    ke = evp.tile([P, QW], BF16, name="ke", tag="qe")
                    nc.scalar.copy(ke[:], kv_ps[:, 0:QW])
                    rope(k_rot[b], boff, ke, toff)
                    ve = evp.tile([P, QW], BF16, name="ve", tag="qe")
                    nc.scalar.copy(ve[:], kv_ps[:, QW:2 * QW])
                    evicted.append(ve)

                def evict_q(h):
                    qe = evp.tile([P, QW], BF16, name="qe", tag="qe")
                    if h < 2:
                        nc.scalar.copy(qe[:], q01[h][:])
                    else:
                        nc.scalar.copy(qe[:], q23_ps[:, (h - 2) * QW:(h - 1) * QW])
                    rope(q_rot[h][b], boff, qe, toff)

                tps = []

                def v_transposes():
                    ve = evicted[0]
                    for i in range(QW // P):
                        tp = ps1.tile([P, P], BF16, name="tp", tag="ps")
                        nc.tensor.transpose(tp[:], ve[:, i * P:(i + 1) * P],
                                            ident[:])
                        tps.append(tp)

                def v_copies(eng=None):
                    for i, tp in enumerate(tps):
                        st = (qx % 2) * 4 + i
                        if eng == "vector":
                            nc.vector.tensor_copy(v_sb[b][:, st, :], tp[:])
                        else:
                            nc.scalar.copy(v_sb[b][:, st, :], tp[:])

                xts = {}
                if qx == 0:
                    # K/V lead Q by QLAG k-tiles so wq k-tiles have an extra
                    # ~12us of HBM-stream slack during the cold start.
                    chunks = [1, 1, 2, 2, 2] + [XTC] * 6
                    k0 = 0
                    qptr = 0
                    for cw in chunks:
                        xt4 = xtp.tile([P, XTC, QW], BF16, name="xt4")
                        nc.sync.dma_start(xt4[:, 0:cw, :],
                                          xt_e[:, qx, k0:k0 + cw, :])
                        for j in range(cw):
                            xts[k0 + j] = xt4[:, j, :]
                            kvmm(k0 + j, xts[k0 + j])
                        k0 += cw
                        # cap the catch-up so a 12-ktile Q tail remains: its
                        # mb-major groups are long enough that each head's
                        # eviction + rope completes during the next group.
                        while qptr <= min(k0 - 1 - QLAG, KT - 13):
                            qmm(qptr, xts.pop(qptr))
                            qptr += 1
                    evict_kv()
                    # Q tail mb-major: early heads stop first so their
                    # evictions + ropes overlap the remaining matmuls.
                    for mb in (2, 3, 4, 5):
                        for kq in range(qptr, KT):
                            mm(mb, kq, xts[kq], kq == 0, kq == KT - 1)
                        evict_q(mb - 2)
                        if mb == 3:
                            v_transposes()
                    v_copies()
                else:
                    chunks = [XTC] * (KT // XTC)
                    k0 = 0
                    tail = []
                    for ci, cw in enumerate(chunks):
                        xt4 = xtp.tile([P, XTC, QW], BF16, name="xt4")
                        nc.sync.dma_start(xt4[:, 0:cw, :],
                                          xt_e[:, qx, k0:k0 + cw, :])
                        if ci < len(chunks) - 3:
                            for j in range(cw):
                                k = k0 + j
                                kvmm(k, xt4[:, j, :])
                                qmm(k, xt4[:, j, :])
                        else:
                            tail += [(k0 + j, xt4[:, j, :]) for j in range(cw)]
                        k0 += cw
                    # 12-ktile tail, mb-major ordered K, q0..q3, V: each
                    # eviction + rope (a ~2.7us scalar+DVE chain) completes
                    # during the following ~2.8us matmul group, so every rope
                    # the next attention phase needs is ready when its first
                    # QK issues.  V runs last: its eviction chain (transposes
                    # + v_sb copies on DVE) only gates the PV matmuls ~2.5us
                    # into the attention phase.
                    for k, xt in tail:
                        mm(0, k, xt, k == 0, k == KT - 1)
                    ke = evp.tile([P, QW], BF16, name="ke", tag="qe")
                    nc.scalar.copy(ke[:], kv_ps[:, 0:QW])
                    rope(k_rot[b], boff, ke, toff)
                    for mb in (2, 3, 4, 5):
                        for k, xt in tail:
                            mm(mb, k, xt, k == 0, k == KT - 1)
                        evict_q(mb - 2)
                    for k, xt in tail:
                        mm(1, k, xt, k == 0, k == KT - 1)
                    ve = evp.tile([P, QW], BF16, name="ve", tag="qe")
                    nc.scalar.copy(ve[:], kv_ps[:, QW:2 * QW])
                    evicted.append(ve)
                    v_transposes()
                    v_copies(eng="vector")

            def attention_batch(b, sqcs=tuple(range(SQC))):
                if variant == "general":
                    attention_batch_general(b, sqcs)
                    return
                for sqc in sqcs:
                    sq0 = sqc * QW
                    stl = _st_info(variant, sqc)
                    # interleave full-width and narrowed st blocks so the
                    # scalar engine's exp stream (cost ~ width) never falls
                    # behind the PE during a run of full-width blocks; keep a
                    # full-width block first (the dacc copy needs full width).
                    full = [e for e in stl if e[1] == 0]
                    narrow = sorted([e for e in stl if e[1] > 0],
                                    key=lambda e: -e[1])
                    stl = []
                    while full or narrow:
                        if full:
                            stl.append(full.pop(0))
                        if narrow:
                            stl.append(narrow.pop())
                    n = len(stl)
                    pairs = ((0, 1), (2, 3))
                    pso = [ps1.tile([P, QW], FP32, name="pso", tag="ps")
                           for _ in range(HPC)]
                    dacc = [dap.tile([P, QW], BF16, name="dacc")
                            for _ in range(HPC)]
                    prs = []

                    def pv(i):
                        st, r, sel = stl[i]
                        for pi, hh in enumerate(pairs):
                            pr = prs[i][pi]
                            for sl, h in enumerate(hh):
                                src = pr[:, r:QW] if sl == 0 \
                                    else pr[:, QW:2 * QW - r]
                                nc.tensor.matmul(pso[h][:, r:QW],
                                                 v_sb[b][:, st, :], src,
                                                 start=(i == 0),
                                                 stop=(i == n - 1))

                    def finish_head(h):
                        # denominator broadcast + normalize + ship to gather
                        psd = ps2.tile([P, 2 * QW], FP32, name="psd",
                                       tag="pair")
                        nc.tensor.matmul(psd[:, 0:QW], ones_sq[:], dacc[h][:],
                                         start=True, stop=True)
                        rb = mip.tile([P, QW], FP32, name="rb")
                        nc.vector.reciprocal_approx_fast(rb[:], psd[:, 0:QW])
                        nc.vector.tensor_mul(attn[h][b][:, sq0:sq0 + QW],
                                             pso[h][:], rb[:])
                        nc.gpsimd.dma_start(
                            ag_in[b][sqc][h * P:(h + 1) * P, :],
                            attn[h][b][:, sq0:sq0 + QW])

                    prs.extend([None, None] for _ in range(n))

                    def emit_pair(i, pi):
                        st, r, sel = stl[i]
                        assert sel is None or sel == r
                        hh = pairs[pi]
                        # both heads of the pair in one 2-bank psum; the
                        # causal-narrowed spans pack adjacently so one exp
                        # covers both heads with no wasted columns:
                        # h0 at [r:QW], h1 at [QW:2QW-r].
                        pss = ps2.tile([P, 2 * QW], FP32, name="pss",
                                       tag="pair")
                        kst = k_rot[b][:, st * P:(st + 1) * P]
                        if sel is not None:
                            # additive causal mask preloaded into the
                            # diagonal span of each head's score bank:
                            # exp then yields exact zeros for future
                            # keys, keeping the PV chain PE+ACT-only.
                            nc.tensor.matmul(pss[:, r:r + P], ident[:],
                                             tri_neg[:],
                                             start=True, stop=False)
                            nc.tensor.matmul(pss[:, QW:QW + P], ident[:],
                                             tri_neg[:],
                                             start=True, stop=False)
                        nc.tensor.matmul(
                            pss[:, r:QW], kst,
                            q_rot[hh[0]][b][:, sq0 + r:sq0 + QW],
                            start=(sel is None), stop=True)
                        nc.tensor.matmul(
                            pss[:, QW:2 * QW - r], kst,
                            q_rot[hh[1]][b][:, sq0 + r:sq0 + QW],
                            start=(sel is None), stop=True)
                        pr = prp.tile([P, 2 * QW], BF16, name="pr",
                                      tag="pr")
                        if i == 0:
                            # per-head exps at the chunk's first block: each
                            # starts one QK earlier and is half as long, so
                            # the second block's QKs (which reuse this psum
                            # slot) stall ~1us less while the exp pipeline
                            # fills.
                            nc.scalar.activation(pr[:, r:QW],
                                                 pss[:, r:QW], EXP)
                            nc.scalar.activation(pr[:, QW:2 * QW - r],
                                                 pss[:, QW:2 * QW - r], EXP)
                        else:
                            nc.scalar.activation(pr[:, r:2 * QW - r],
                                                 pss[:, r:2 * QW - r], EXP)
                        # denominator partial sums on DVE (bf16); only
                        # consumed by the ones-matmul at chunk end, so
                        # DVE lag never stalls the PE.  The last two blocks'
                        # adds are deferred and interleaved per-head with
                        # finish_head so recip(h0) (whose completion
                        # releases the next proj phase's PSUM banks via
                        # WAR) runs as early as possible.
                        if i < n - 2:
                            for sl, h in enumerate(hh):
                                src = pr[:, r:QW] if sl == 0 \
                                    else pr[:, QW:2 * QW - r]
                                if i == 0:
                                    nc.vector.tensor_copy(dacc[h][:], src)
                                else:
                                    nc.vector.tensor_add(
                                        dacc[h][:, r:QW],
                                        dacc[h][:, r:QW], src)
                        prs[i][pi] = pr

                    for i in range(n):
                        emit_pair(i, 0)
                        emit_pair(i, 1)
                        if i >= 1:
                            pv(i - 1)
                    pv(n - 1)
                    for h in range(HPC):
                        for i in (n - 2, n - 1):
                            st, r, sel = stl[i]
                            pr = prs[i][h // 2]
                            src = pr[:, r:QW] if h % 2 == 0 \
                                else pr[:, QW:2 * QW - r]
                            nc.vector.tensor_add(dacc[h][:, r:QW],
                                                 dacc[h][:, r:QW], src)
                        finish_head(h)

            def attention_batch_general(b, sqcs):
                for sqc in sqcs:
                    sq0 = sqc * QW
                    stl = _st_info(variant, sqc)
                    n = len(stl)
                    for h in range(HPC):
                        prtiles = []
                        for st, r, sel in stl:
                            pss = ps1.tile([P, QW], FP32, name="pss", tag="ps")
                            nc.tensor.matmul(pss[:], ident[:],
                                             mk_sb[:, st, sq0:sq0 + QW],
                                             start=True, stop=False)
                            nc.tensor.matmul(
                                pss[:], k_rot[b][:, st * P:(st + 1) * P],
                                q_rot[h][b][:, sq0:sq0 + QW],
                                start=False, stop=True)
                            pr = prp.tile([P, QW], BF16, name="pr", tag="pr")
                            nc.scalar.activation(pr[:], pss[:], EXP)
                            prtiles.append(pr)
                        pso = ps2.tile([P, 2 * QW], FP32, name="psog",
                                       tag="pair")
                        for i, (st, r, sel) in enumerate(stl):
                            pr = prtiles[i]
                            nc.tensor.matmul(pso[:, 0:QW], v_sb[b][:, st, :],
                                             pr[:],
                                             start=(i == 0), stop=(i == n - 1))
                            nc.tensor.matmul(pso[:, QW:2 * QW], ones_sq[:],
                                             pr[:],
                                             start=(i == 0), stop=(i == n - 1))
                        rb = mip.tile([P, QW], FP32, name="rb")
                        nc.vector.reciprocal_approx_fast(rb[:],
                                                         pso[:, QW:2 * QW])
                        nc.vector.tensor_mul(attn[h][b][:, sq0:sq0 + QW],
                                             pso[:, 0:QW], rb[:])
                        nc.gpsimd.dma_start(
                            ag_in[b][sqc][h * P:(h + 1) * P, :],
                            attn[h][b][:, sq0:sq0 + QW])

            def gather_batch(b, c):
                nc.gpsimd.collective_compute(
                    "AllGather", mybir.AluOpType.bypass,
                    ins=[ag_in[b][c][:].opt()],
                    outs=[ag_out[b][c][:].opt()],
                    replica_groups=[list(range(NCORES))],
                )

            def wo_chunk(b, nch, last):
                """wo matmuls for one (batch, sq-chunk): full 4096-contraction
                over 512 tokens.  Processing sq-chunks serially (not
                interleaved) means chunk 0's matmuls only need that chunk's
                AllGather -- chunk 1's gather (the last collective for the
                batch) gets an extra ~35us of slack before first use."""
                ag_r = ag_out[b][nch].rearrange("(k p) t -> p k t", p=P)
                psw_pair = ps2.tile([P, 2 * QW], FP32, name="psw_pair",
                                    tag="pair")
                psw_s = [ps1.tile([P, QW], FP32, name="psw", tag="ps")
                         for _ in range(2)]

                def psw(mb):
                    if mb < 2:
                        return psw_pair[:, mb * QW:(mb + 1) * QW]
                    return psw_s[mb - 2][:]

                nchk = KT // AGC
                wtail = []
                for kc in range(nchk):
                    agt = agp.tile([P, AGC, QW], BF16, name="agt")
                    nc.sync.dma_start(agt[:],
                                      ag_r[:, kc * AGC:(kc + 1) * AGC, :])
                    if kc < nchk - 2:
                        for j in range(AGC):
                            k = kc * AGC + j
                            for mb in range(4):
                                w = wq_sb[:, k, mb * P:(mb + 1) * P]
                                nc.tensor.matmul(
                                    psw(mb), w, agt[:, j, :],
                                    start=(k == 0), stop=False)
                    else:
                        wtail += [(kc * AGC + j, agt) for j in range(AGC)]
                # last two chunks mb-major so early mb groups stop several us
                # before the end and their evictions + out DMAs overlap the
                # remaining matmuls.
                tcol = b * S + nch * QW
                for mb in range(4):
                    for k, agt in wtail:
                        w = wq_sb[:, k, mb * P:(mb + 1) * P]
                        nc.tensor.matmul(
                            psw(mb), w, agt[:, k % AGC, :],
                            start=False, stop=(k == KT - 1))
                    if mb == 3 and last:
                        # split the very last eviction in half across
                        # engines/queues so its copy + DMA pipeline
                        # instead of serializing after the final matmul.
                        hw = QW // 2
                        for hf in range(2):
                            ow = owp.tile([P, hw], BF16, name="owh",
                                          tag="owh")
                            src = psw(mb)[:, hf * hw:(hf + 1) * hw]
                            if hf == 0:
                                nc.vector.tensor_copy(ow[:], src)
                                dma = nc.sync.dma_start
                            else:
                                nc.scalar.copy(ow[:], src)
                                dma = nc.scalar.dma_start
                            dma(out_e[mb * P:(mb + 1) * P,
                                      tcol + hf * hw:tcol + (hf + 1) * hw],
                                ow[:])
                        continue
                    ow = owp.tile([P, QW], BF16, name="ow")
                    if mb % 2 == 0:
                        nc.scalar.copy(ow[:], psw(mb))
                        dma = nc.scalar.dma_start
                    else:
                        nc.vector.tensor_copy(ow[:], psw(mb))
                        dma = nc.sync.dma_start
                    dma(out_e[mb * P:(mb + 1) * P, tcol:tcol + QW], ow[:])

            def wo_batch(b):
                for nch in range(SQC):
                    wo_chunk(b, nch, last=(b == B - 1 and nch == SQC - 1))

            # ---- timeline ----
            # causal: attention sq-chunks interleave between proj quarters --
            # chunk s0 of batch b only needs that batch's first token quarter
            # (no future keys).  This fires gather(0) earlier and gives the
            # xt/weight streams HBM-quiet windows (attention phases do no HBM
            # traffic) to get ahead.  Non-causal variants attend future keys,
            # so each batch's attention must wait for BOTH its quarters.
            load_weights()
            if variant == "causal":
                proj_quarter(0)
                attention_batch(0, (0,))
                gather_batch(0, 0)
                proj_quarter(1)
                attention_batch(0, (1,))
                gather_batch(0, 1)
                proj_quarter(2)
                attention_batch(1, (0,))
                gather_batch(1, 0)
                proj_quarter(3)
                wo_load()
                attention_batch(1, (1,))
                gather_batch(1, 1)
            else:
                proj_quarter(0)
                proj_quarter(1)
                attention_batch(0, (0,))
                gather_batch(0, 0)
                attention_batch(0, (1,))
                gather_batch(0, 1)
                proj_quarter(2)
                proj_quarter(3)
                wo_load()
                attention_batch(1, (0,))
                gather_batch(1, 0)
                attention_batch(1, (1,))
                gather_batch(1, 1)
            wo_batch(0)
            wo_batch(1)

    nc.compile()
    return nc


def _get_compiled(variant):
    if variant not in _COMPILED:
        _COMPILED[variant] = _build(variant)
    return _COMPILED[variant]


def _detect_variant(mask2d):
    if not np.any(mask2d):
        return "nomask"
    tril = np.tril(mask2d)
    if not np.any(tril):
        iu = np.triu_indices(S, 1)
        if np.all(mask2d[iu] <= -1e8):
            return "causal"
    return "general"


def _pack_kt(w):
    """[R*128, N] -> [128, R, N] so that [:, k, :] is rows k*128..k*128+127."""
    return np.ascontiguousarray(w.reshape(w.shape[0] // P, P, -1).transpose(1, 0, 2))


def kernel(x, wq, wk, wv, wo, lora_q_a, lora_q_b, lora_v_a, lora_v_b,
           freqs_cos, freqs_sin, mask, start_pos=0, **_):
    global LAST_RESULTS
    bf = ml_dtypes.bfloat16
    x = np.asarray(x, np.float32)
    wq = np.asarray(wq, np.float32)
    wk = np.asarray(wk, np.float32)
    wv = np.asarray(wv, np.float32)
    wo = np.asarray(wo, np.float32)
    lora_q_a = np.asarray(lora_q_a, np.float32)
    lora_q_b = np.asarray(lora_q_b, np.float32)
    lora_v_a = np.asarray(lora_v_a, np.float32)
    lora_v_b = np.asarray(lora_v_b, np.float32)
    cos = np.asarray(freqs_cos, np.float32)
    sin = np.asarray(freqs_sin, np.float32)
    mask2d = np.asarray(mask, np.float32).reshape(S, S)

    variant = _detect_variant(mask2d)
    nc = _get_compiled(variant)

    # fold LoRA + scale; permute rope pairs (evens then odds within each head)
    wq_eff = (wq + lora_q_a @ lora_q_b) * np.float32(1.0 / np.sqrt(HD))
    wv_eff = wv + lora_v_a @ lora_v_b
    perm = np.concatenate([np.arange(0, HD, 2), np.arange(1, HD, 2)])
    qperm = (np.arange(H)[:, None] * HD + perm[None, :]).reshape(-1)
    kperm = (np.arange(KVH)[:, None] * HD + perm[None, :]).reshape(-1)
    wq_eff = wq_eff[:, qperm]
    wk_p = wk[:, kperm]

    xt = np.ascontiguousarray(x.reshape(T, D).T)        # [4096, 2048]
    # [128, KT, T] -> quarter-major [128, NQ, KT, QW] (contiguous per chunk)
    xt_p = np.ascontiguousarray(
        _pack_kt(xt).reshape(P, KT, NQ, QW).transpose(0, 2, 1, 3)).astype(bf)
    c64 = np.tile(cos.T, (1, B))                        # [64, 2048]
    s64 = np.tile(sin.T, (1, B))
    cosT = np.concatenate([c64, c64], axis=0).astype(bf)   # [c; c]
    sinT = np.concatenate([s64, -s64], axis=0).astype(bf)  # [s; -s]

    if variant == "general":
        maskT = np.ascontiguousarray(mask2d.T)          # [st, sq]
        mk = _pack_kt(maskT).astype(bf)                 # [128, 8, 1024]
    else:
        mk = None

    in_maps = []
    for c in range(NCORES):
        im = {
            "xt": xt_p,
            "wq": _pack_kt(wq_eff[:, c * QCOLS:(c + 1) * QCOLS]).astype(bf),
            "wk": _pack_kt(wk_p[:, c * HD:(c + 1) * HD]).astype(bf),
            "wv": _pack_kt(wv_eff[:, c * HD:(c + 1) * HD]).astype(bf),
            "wo": _pack_kt(wo[:, c * QCOLS:(c + 1) * QCOLS]).astype(bf),
            "cos": cosT,
            "sin": sinT,
        }
        if mk is not None:
            im["mk"] = mk
        in_maps.append(im)

    res = run_bass_kernel_spmd(nc, in_maps, core_ids=list(range(NCORES)))
    LAST_RESULTS = res
    outT = np.concatenate([res.results[c]["out"] for c in range(NCORES)], axis=0)
    return np.ascontiguousarray(outT.T).reshape(B, S, D).astype(np.float32)


# revision 39
# speedup vs baseline: 1.0291x; 1.0133x over previous
"""Distributed Trainium2 Bass kernel for nn_Attention (GQA attention + LoRA + RoPE).

Sharding: tensor-parallel over heads across 8 NeuronCores.
  - core c owns Q heads 4c..4c+3 and KV head c (GQA group).
  - wq/wk/wv column-sharded; wo COLUMN-sharded (each core computes a
    512-column slice of the output over the full 4096 contraction, fed by an
    AllGather of all cores' per-head attention outputs).
  - LoRA is folded into wq/wv on the host (x@wq + (x@A)@B == x@(wq + A@B)).
  - 1/sqrt(HD) folded into wq.
  - RoPE pair permutation folded into wq/wk column order: within each head the
    even dims come first, odd dims second, so on-device RoPE is plain
    elementwise math on partition halves.

Everything the device computes is bf16-in/f32-accumulate.

v3 performance notes (vs the 432us v2):
  The chip runs GPIO-power-throttled to 13/16 (~1.95GHz PE) for ~90% of the
  kernel, so v3 attacks streamed-column count and non-PE stalls:
  - softmax denominator no longer uses per-st ones-matmuls: pr tiles are
    summed on DVE (bf16, lag-tolerant) and a single ones-matmul per
    (head, chunk) broadcasts it (-28.7K PE columns).  The last two blocks'
    adds interleave per-head with the finish chain so the next phase's
    PSUM WARs release early.
  - QK head-pairs pack into one 2-bank PSUM tile with the causal-narrowed
    spans adjacent, so ONE exp ACTIVATE covers two heads with no wasted
    columns (scalar exp time 58us -> 22us); the first block uses per-head
    exps to prime the pipeline.
  - causal masking preloaded as a -1e9 lower-tri matmul into the diagonal
    span of the score psum (exp yields exact zeros): the QK->exp->PV chain
    touches only PE+ACT, never DVE/gpsimd.
  - PV matmuls lag QK by one st block so exp latency never stalls the PE;
    full/narrow st blocks interleave so exp cost tracks PE cost.
  - proj quarter 0 runs K/V matmuls 8 k-tiles ahead of Q matmuls with the
    weight stream ordered to match (kills a 7.5us HBM-starvation stall);
    quarters 1-3 defer a 12-ktile tail ordered K,q0..q3,V so every rope
    the next attention phase needs completes during the tail (4-op ropes,
    v_sb copies on DVE).
  - wo processes each (batch, sq-chunk) serially over the full contraction,
    so the last AllGather gets ~35us of slack before first use (collective
    skew no longer stalls the PE); last eviction split across engines.
  - PSUM: 2-bank pair tiles (kv / q23 in proj, QK pairs in attention,
    2 of 4 wo accumulators) + 4 single banks, exactly filling 8 banks.
  - non-causal masks attend future keys, so those variants run each batch's
    attention only after BOTH its token quarters are projected (the v2
    schedule read uninitialized k/v for them).
"""

import sys
import types

import numpy as np
import ml_dtypes

import concourse.bass as bass
from concourse import bacc
import concourse.mybir as mybir
import concourse.tile as tile
from concourse.bass_utils import run_bass_kernel_spmd
from concourse.masks import make_identity


def _ensure_axon_hooks():
    """run_bass_kernel_spmd(trace=True) imports antenv.axon_hooks, which some
    images lack; install a no-op shim so a BASS_TRACE env var can't crash us."""
    try:
        import antenv
    except ImportError:
        return
    if "antenv.axon_hooks" in sys.modules:
        return
    try:
        from antenv import axon_hooks  # noqa: F401
        return
    except ImportError:
        pass
    mod = types.ModuleType("antenv.axon_hooks")
    mod._hook = None
    mod.set_axon_ntff_profile_hook = lambda h: setattr(mod, "_hook", h)
    mod.get_axon_ntff_profile_hook = lambda: mod._hook
    sys.modules["antenv.axon_hooks"] = mod
    antenv.axon_hooks = mod


_ensure_axon_hooks()

B, S, D = 2, 1024, 4096
H, KVH, HD = 32, 8, 128
NCORES = 8
HPC = H // NCORES            # 4 q heads per core
QCOLS = HPC * HD             # 512
T = B * S                    # 2048
P = 128
KT = D // P                  # 32 k tiles
NQ = 4                       # token quarters (512 tokens each)
QW = T // NQ                 # 512
SQC = 2                      # sq chunks per batch
STB = S // P                 # 8 st blocks per batch
XTC = 4                      # k-tiles per xt DMA chunk
AGC = 2                      # k-tiles per allgather-readback DMA chunk
QLAG = 8                     # quarter-0 K/V lead over Q, in k-tiles

FP32 = mybir.dt.float32
BF16 = mybir.dt.bfloat16
EXP = mybir.ActivationFunctionType.Exp

_COMPILED = {}
LAST_RESULTS = None


def _st_info(variant, sqc):
    """st blocks contributing to sq chunk sqc, as (st, r, sel):
    r = first needed column within the 512-wide chunk (0 for full width),
    sel = start of the 128-wide diagonal span needing triangular zeroing
    (None if the block is fully below the diagonal / no mask)."""
    out = []
    for st in range(STB):
        if variant == "causal":
            rd = st * P - sqc * QW
            if rd >= QW:
                continue  # fully masked
            if rd >= 0:
                out.append((st, rd, rd))
            else:
                out.append((st, 0, None))
        else:
            out.append((st, 0, None))
    return out


def _build(variant):
    nc = bacc.Bacc(None)

    # xt packed quarter-major: [:, qx, k, :] is per-partition contiguous 4KB
    # per 4-ktile chunk, so xt chunk DMAs run at large-descriptor efficiency.
    xt_e = nc.declare_dram_parameter("xt", [P, NQ, KT, QW], BF16, isOutput=False)
    wq_e = nc.declare_dram_parameter("wq", [P, KT, QCOLS], BF16, isOutput=False)
    wk_e = nc.declare_dram_parameter("wk", [P, KT, HD], BF16, isOutput=False)
    wv_e = nc.declare_dram_parameter("wv", [P, KT, HD], BF16, isOutput=False)
    wo_e = nc.declare_dram_parameter("wo", [P, KT, QCOLS], BF16, isOutput=False)
    # cos: [c; c] duplicated halves.  sin: [s; -s] (negated bottom half).
    cos_e = nc.declare_dram_parameter("cos", [P, T], BF16, isOutput=False)
    sin_e = nc.declare_dram_parameter("sin", [P, T], BF16, isOutput=False)
    if variant == "general":
        mk_e = nc.declare_dram_parameter("mk", [P, STB, S], BF16, isOutput=False)
    out_e = nc.declare_dram_parameter("out", [QCOLS, T], BF16, isOutput=True)

    with tile.TileContext(nc) as tc:
        with (
            tc.tile_pool(name="wpool", bufs=1) as wpool,
            tc.tile_pool(name="cst", bufs=1) as cst,
            tc.tile_pool(name="persist", bufs=1) as persist,
            tc.tile_pool(name="xt", bufs=10 if variant != "general" else 8) as xtp,
            tc.tile_pool(name="ev", bufs=4) as evp,
            tc.tile_pool(name="rt", bufs=3) as rtp,
            tc.tile_pool(name="probs", bufs=7 if variant != "general" else 20) as prp,
            tc.tile_pool(name="dacc", bufs=4) as dap,
            tc.tile_pool(name="misc", bufs=3) as mip,
            tc.tile_pool(name="ag", bufs=12 if variant != "general" else 6) as agp,
            tc.tile_pool(name="ow", bufs=8) as owp,
            tc.tile_pool(name="ps2", bufs=2, space="PSUM") as ps2,
            tc.tile_pool(name="ps1", bufs=4, space="PSUM") as ps1,
            tc.tile_pool(name="dram", bufs=1, space="DRAM") as dram,
        ):
            # ---- resident weights / constants ----
            # wq_sb doubles as wo storage: wo is DMA'd over it after proj
            # quarter 3's last wq read (Tile WAR tracking orders this).
            wq_sb = wpool.tile([P, KT, QCOLS], BF16, name="wq_sb")
            wk_sb = wpool.tile([P, KT, HD], BF16, name="wk_sb")
            wv_sb = wpool.tile([P, KT, HD], BF16, name="wv_sb")
            cos_sb = wpool.tile([P, T], BF16, name="cos_sb")
            sin_sb = wpool.tile([P, T], BF16, name="sin_sb")
            if variant == "general":
                mk_sb = wpool.tile([P, STB, S], BF16, name="mk_sb")

            ident = cst.tile([P, P], BF16, name="ident")
            make_identity(nc, ident)
            ones_sq = cst.tile([P, P], BF16, name="ones_sq")
            nc.vector.memset(ones_sq[:], 1.0)
            # additive causal mask for a diagonal 128x128 span:
            # tri_neg[p, j] = 0 if p <= j else -1e9 (future keys killed
            # pre-exp, so no post-exp masking op is needed anywhere).
            zeros_sq = cst.tile([P, P], BF16, name="zeros_sq")
            nc.vector.memset(zeros_sq[:], 0.0)
            tri_neg = cst.tile([P, P], BF16, name="tri_neg")
            nc.gpsimd.affine_select(
                out=tri_neg[:], in_=zeros_sq[:],
                compare_op=mybir.AluOpType.is_ge, fill=-1e9,
                base=0, channel_multiplier=-1, pattern=[[1, P]])

            # ---- persistent activations ----
            q_rot = [[persist.tile([P, S], BF16, name=f"q{h}_{b}")
                      for b in range(B)] for h in range(HPC)]
            k_rot = [persist.tile([P, S], BF16, name=f"k{b}") for b in range(B)]
            v_sb = [persist.tile([P, STB, P], BF16, name=f"v{b}") for b in range(B)]
            attn = [[persist.tile([P, S], BF16, name=f"attn{h}_{b}")
                     for b in range(B)] for h in range(HPC)]

            # per-(batch, sq-half) gather buffers: two smaller collectives per
            # batch, each fired as soon as its attention chunk finishes --
            # spreads collective traffic and halves skew exposure.
            ag_in = [[dram.tile([HPC * P, QW], BF16, name=f"agin{b}_{c}")
                      for c in range(SQC)] for b in range(B)]
            ag_out = [[dram.tile([H * P, QW], BF16, addr_space="Shared",
                                 name=f"agout{b}_{c}") for c in range(SQC)]
                      for b in range(B)]

            def rope(dst, dst_off, src_bf, qoff):
                """RoPE on split layout (a=0:64, b=64:128), 4 DVE ops.
                p1 = [a*c; b*c];  p2sw = [b*(-s); a*s] computed directly with
                cross-partition reads (sin table already holds [s; -s]);
                dst = p1 + p2sw = [a*c - b*s; a*s + b*c]."""
                c = cos_sb[:, qoff:qoff + QW]
                p1 = rtp.tile([P, QW], BF16, name="p1")
                p2sw = rtp.tile([P, QW], BF16, name="p2sw")
                nc.vector.tensor_mul(p1[:], src_bf[:], c)
                nc.vector.tensor_mul(p2sw[0:64, :], src_bf[64:128, :],
                                     sin_sb[64:128, qoff:qoff + QW])
                nc.vector.tensor_mul(p2sw[64:128, :], src_bf[0:64, :],
                                     sin_sb[0:64, qoff:qoff + QW])
                nc.vector.tensor_add(dst[:, dst_off:dst_off + QW], p1[:], p2sw[:])

            def load_weights():
                """Stream projection weights + rope tables on the scalar
                (HWDGE) queue, ordered by first consumption under the
                quarter-0 schedule (K/V leading Q by QLAG k-tiles), so the
                sync queue only carries the xt stream and no weight arrives
                later than the matmul that needs it."""
                dma = nc.scalar.dma_start

                def tables(qx):
                    toff = qx * QW
                    dma(cos_sb[:, toff:toff + QW], cos_e[:, toff:toff + QW])
                    dma(sin_sb[:, toff:toff + QW], sin_e[:, toff:toff + QW])

                # single-ktile leading chunks: K(0)/V(0) matmuls unlock after
                # 32KB each instead of 128KB during the cold-HBM ramp.
                dma(wk_sb[:, 0:1, :], wk_e[:, 0:1, :])
                dma(wv_sb[:, 0:1, :], wv_e[:, 0:1, :])
                dma(wk_sb[:, 1:2, :], wk_e[:, 1:2, :])
                dma(wv_sb[:, 1:2, :], wv_e[:, 1:2, :])
                dma(wk_sb[:, 2:8, :], wk_e[:, 2:8, :])
                dma(wv_sb[:, 2:8, :], wv_e[:, 2:8, :])
                dma(wk_sb[:, 8:16, :], wk_e[:, 8:16, :])
                dma(wv_sb[:, 8:16, :], wv_e[:, 8:16, :])
                dma(wq_sb[:, 0:4, :], wq_e[:, 0:4, :])
                dma(wk_sb[:, 16:24, :], wk_e[:, 16:24, :])
                dma(wv_sb[:, 16:24, :], wv_e[:, 16:24, :])
                dma(wq_sb[:, 4:8, :], wq_e[:, 4:8, :])
                dma(wq_sb[:, 8:12, :], wq_e[:, 8:12, :])
                dma(wk_sb[:, 24:KT, :], wk_e[:, 24:KT, :])
                dma(wv_sb[:, 24:KT, :], wv_e[:, 24:KT, :])
                tables(0)
                dma(wq_sb[:, 12:16, :], wq_e[:, 12:16, :])
                dma(wq_sb[:, 16:20, :], wq_e[:, 16:20, :])
                tables(1)
                dma(wq_sb[:, 20:24, :], wq_e[:, 20:24, :])
                dma(wq_sb[:, 24:28, :], wq_e[:, 24:28, :])
                dma(wq_sb[:, 28:KT, :], wq_e[:, 28:KT, :])
                tables(2)
                tables(3)
                if variant == "general":
                    dma(mk_sb[:], mk_e[:])

            def wo_load():
                """Overwrite wq_sb with wo (WAR-ordered after the last wq read,
                i.e. streams during attention_batch(1)).  On the scalar queue so
                the sync queue only carries agt readback for wo_batch(0)."""
                for c in range(4):
                    nc.scalar.dma_start(wq_sb[:, 8 * c:8 * c + 8, :],
                                        wo_e[:, 8 * c:8 * c + 8, :])

            def proj_quarter(qx):
                b, boff = qx // 2, (qx % 2) * QW
                toff = qx * QW
                # psums: kv pair (k | v), q0, q1 single banks, q23 pair.
                kv_ps = ps2.tile([P, 2 * QW], FP32, name="kv_ps", tag="pair")
                q01 = [ps1.tile([P, QW], FP32, name="q01", tag="ps")
                       for _ in range(2)]
                q23_ps = ps2.tile([P, 2 * QW], FP32, name="q23_ps", tag="pair")

                def mm(mb, k, xt, start, stop):
                    if mb == 0:
                        w, dst = wk_sb[:, k, :], kv_ps[:, 0:QW]
                    elif mb == 1:
                        w, dst = wv_sb[:, k, :], kv_ps[:, QW:2 * QW]
                    elif mb < 4:
                        w = wq_sb[:, k, (mb - 2) * P:(mb - 1) * P]
                        dst = q01[mb - 2][:]
                    else:
                        w = wq_sb[:, k, (mb - 2) * P:(mb - 1) * P]
                        dst = q23_ps[:, (mb - 4) * QW:(mb - 3) * QW]
                    nc.tensor.matmul(dst, w, xt, start=start, stop=stop)

                def kvmm(k, xt):
                    mm(0, k, xt, k == 0, k == KT - 1)
                    mm(1, k, xt, k == 0, k == KT - 1)

                def qmm(k, xt, mbs=(2, 3, 4, 5)):
                    for mb in mbs:
                        mm(mb, k, xt, k == 0, k == KT - 1)

                evicted = []

                def evict_kv():
                    ke = evp.tile([P, QW], BF16, name="ke", tag="qe")
                    nc.scalar.copy(ke[:], kv_ps[:, 0:QW])
                    rope(k_rot[b], boff, ke, toff)
                    ve = evp.tile([P, QW], BF16, name="ve", tag="qe")
                    nc.scalar.copy(ve[:], kv_ps[:, QW:2 * QW])
                    evicted.append(ve)

                def evict_q(h):
                    qe = evp.tile([P, QW], BF16, name="qe", tag="qe")
                    if h < 2:
                        nc.scalar.copy(qe[:], q01[h][:])
                    else:
                        nc.scalar.copy(qe[:], q23_ps[:, (h - 2) * QW:(h - 1) * QW])
                    rope(q_rot[h][b], boff, qe, toff)

                tps = []

                def v_transposes():
                    ve = evicted[0]
                    for i in range(QW // P):
                        tp = ps1.tile([P, P], BF16, name="tp", tag="ps")
                        nc.tensor.transpose(tp[:], ve[:, i * P:(i + 1) * P],
                                            ident[:])
                        tps.append(tp)

                def v_copies(eng=None):
                    for i, tp in enumerate(tps):
                        st = (qx % 2) * 4 + i
                        if eng == "vector":
                            nc.vector.tensor_copy(v_sb[b][:, st, :], tp[:])
                        else:
                            nc.scalar.copy(v_sb[b][:, st, :], tp[:])

                xts = {}
                if qx == 0:
                    # K/V lead Q by QLAG k-tiles so wq k-tiles have an extra
                    # ~12us of HBM-stream slack during the cold start.
                    chunks = [1, 1, 2, 2, 2] + [XTC] * 6
                    k0 = 0
                    qptr = 0
                    for cw in chunks:
                        xt4 = xtp.tile([P, XTC, QW], BF16, name="xt4")
                        nc.sync.dma_start(xt4[:, 0:cw, :],
                                          xt_e[:, qx, k0:k0 + cw, :])
                        for j in range(cw):
                            xts[k0 + j] = xt4[:, j, :]
                            kvmm(k0 + j, xts[k0 + j])
                        k0 += cw
                        # cap the catch-up so a 12-ktile Q tail remains: its
                        # mb-major groups are long enough that each head's
                        # eviction + rope completes during the next group.
                        while qptr <= min(k0 - 1 - QLAG, KT - 13):
                            qmm(qptr, xts.pop(qptr))
                            qptr += 1
                    evict_kv()
                    # Q tail mb-major: early heads stop first so their
                    # evictions + ropes overlap the remaining matmuls.
                    for mb in (2, 3, 4, 5):
                        for kq in range(qptr, KT):
                            mm(mb, kq, xts[kq], kq == 0, kq == KT - 1)
                        evict_q(mb - 2)
                        if mb == 3:
                            v_transposes()
                    v_copies()
                else:
                    chunks = [XTC] * (KT // XTC)
                    k0 = 0
                    tail = []
                    for ci, cw in enumerate(chunks):
                        xt4 = xtp.tile([P, XTC, QW], BF16, name="xt4")
                        nc.sync.dma_start(xt4[:, 0:cw, :],
                                          xt_e[:, qx, k0:k0 + cw, :])
                        if ci < len(chunks) - 3:
                            for j in range(cw):
                                k = k0 + j
                                kvmm(k, xt4[:, j, :])
                                qmm(k, xt4[:, j, :])
                        else:
                            tail += [(k0 + j, xt4[:, j, :]) for j in range(cw)]
                        k0 += cw
                    # 12-ktile tail, mb-major ordered K, q0..q3, V: each
                    # eviction + rope (a ~2.7us scalar+DVE chain) completes
                    # during the following ~2.8us matmul group, so every rope
                    # the next attention phase needs is ready when its first
                    # QK issues.  V runs last: its eviction chain (transposes
                    # + v_sb copies on DVE) only gates the PV matmuls ~2.5us
                    # into the attention phase.
                    for k, xt in tail:
                        mm(0, k, xt, k == 0, k == KT - 1)
                    ke = evp.tile([P, QW], BF16, name="ke", tag="qe")
                    nc.scalar.copy(ke[:], kv_ps[:, 0:QW])
                    rope(k_rot[b], boff, ke, toff)
                    for mb in (2, 3, 4, 5):
                        for k, xt in tail:
                            mm(mb, k, xt, k == 0, k == KT - 1)
                        evict_q(mb - 2)
                    for k, xt in tail:
                        mm(1, k, xt, k == 0, k == KT - 1)
                    ve = evp.tile([P, QW], BF16, name="ve", tag="qe")
                    nc.scalar.copy(ve[:], kv_ps[:, QW:2 * QW])
                    evicted.append(ve)
                    v_transposes()
                    v_copies(eng="vector")

            def attention_batch(b, sqcs=tuple(range(SQC))):
                if variant == "general":
                    attention_batch_general(b, sqcs)
                    return
                for sqc in sqcs:
                    sq0 = sqc * QW
                    stl = _st_info(variant, sqc)
                    # interleave full-width and narrowed st blocks so the
                    # scalar engine's exp stream (cost ~ width) never falls
                    # behind the PE during a run of full-width blocks; keep a
                    # full-width block first (the dacc copy needs full width)
                    # and the narrowest LAST so the window tail (exp -> dacc
                    # -> ones-matmul -> recip chain) is as short as possible.
                    full = [e for e in stl if e[1] == 0]
                    narrow = sorted([e for e in stl if e[1] > 0],
                                    key=lambda e: e[1])  # widest first
                    stl = []
                    while full:
                        stl.append(full.pop(0))
                        if len(narrow) > 1:
                            stl.append(narrow.pop(0))
                    stl.extend(narrow)
                    n = len(stl)
                    pairs = ((0, 1), (2, 3))
                    pso = [ps1.tile([P, QW], FP32, name="pso", tag="ps")
                           for _ in range(HPC)]
                    dacc = [dap.tile([P, QW], BF16, name="dacc")
                            for _ in range(HPC)]
                    prs = []

                    def pv(i):
                        st, r, sel = stl[i]
                        for pi, hh in enumerate(pairs):
                            pr = prs[i][pi]
                            for sl, h in enumerate(hh):
                                src = pr[:, r:QW] if sl == 0 \
                                    else pr[:, QW:2 * QW - r]
                                nc.tensor.matmul(pso[h][:, r:QW],
                                                 v_sb[b][:, st, :], src,
                                                 start=(i == 0),
                                                 stop=(i == n - 1))

                    def finish_head(h):
                        # denominator broadcast + normalize + ship to gather
                        psd = ps2.tile([P, 2 * QW], FP32, name="psd",
                                       tag="pair")
                        nc.tensor.matmul(psd[:, 0:QW], ones_sq[:], dacc[h][:],
                                         start=True, stop=True)
                        rb = mip.tile([P, QW], FP32, name="rb")
                        nc.vector.reciprocal_approx_fast(rb[:], psd[:, 0:QW])
                        nc.vector.tensor_mul(attn[h][b][:, sq0:sq0 + QW],
                                             pso[h][:], rb[:])
                        nc.gpsimd.dma_start(
                            ag_in[b][sqc][h * P:(h + 1) * P, :],
                            attn[h][b][:, sq0:sq0 + QW])

                    prs.extend([None, None] for _ in range(n))

                    def emit_pair(i, pi):
                        st, r, sel = stl[i]
                        assert sel is None or sel == r
                        hh = pairs[pi]
                        # both heads of the pair in one 2-bank psum; the
                        # causal-narrowed spans pack adjacently so one exp
                        # covers both heads with no wasted columns:
                        # h0 at [r:QW], h1 at [QW:2QW-r].
                        pss = ps2.tile([P, 2 * QW], FP32, name="pss",
                                       tag="pair")
                        kst = k_rot[b][:, st * P:(st + 1) * P]
                        if sel is not None:
                            # additive causal mask preloaded into the
                            # diagonal span of each head's score bank:
                            # exp then yields exact zeros for future
                            # keys, keeping the PV chain PE+ACT-only.
                            nc.tensor.matmul(pss[:, r:r + P], ident[:],
                                             tri_neg[:],
                                             start=True, stop=False)
                            nc.tensor.matmul(pss[:, QW:QW + P], ident[:],
                                             tri_neg[:],
                                             start=True, stop=False)
                        nc.tensor.matmul(
                            pss[:, r:QW], kst,
                            q_rot[hh[0]][b][:, sq0 + r:sq0 + QW],
                            start=(sel is None), stop=True)
                        nc.tensor.matmul(
                            pss[:, QW:2 * QW - r], kst,
                            q_rot[hh[1]][b][:, sq0 + r:sq0 + QW],
                            start=(sel is None), stop=True)
                        pr = prp.tile([P, 2 * QW], BF16, name="pr",
                                      tag="pr")
                        if i == 0:
                            # per-head exps at the chunk's first block: each
                            # starts one QK earlier and is half as long, so
                            # the second block's QKs (which reuse this psum
                            # slot) stall ~1us less while the exp pipeline
                            # fills.
                            nc.scalar.activation(pr[:, r:QW],
                                                 pss[:, r:QW], EXP)
                            nc.scalar.activation(pr[:, QW:2 * QW - r],
                                                 pss[:, QW:2 * QW - r], EXP)
                        else:
                            nc.scalar.activation(pr[:, r:2 * QW - r],
                                                 pss[:, r:2 * QW - r], EXP)
                        # denominator partial sums on DVE (bf16); only
                        # consumed by the ones-matmul at chunk end, so
                        # DVE lag never stalls the PE.  The last two blocks'
                        # adds are deferred and interleaved per-head with
                        # finish_head so recip(h0) (whose completion
                        # releases the next proj phase's PSUM banks via
                        # WAR) runs as early as possible.
                        if i < n - 2:
                            for sl, h in enumerate(hh):
                                src = pr[:, r:QW] if sl == 0 \
                                    else pr[:, QW:2 * QW - r]
                                if i == 0:
                                    nc.vector.tensor_copy(dacc[h][:], src)
                                else:
                                    nc.vector.tensor_add(
                                        dacc[h][:, r:QW],
                                        dacc[h][:, r:QW], src)
                        prs[i][pi] = pr

                    for i in range(n):
                        emit_pair(i, 0)
                        emit_pair(i, 1)
                        if i >= 1:
                            pv(i - 1)
                    pv(n - 1)
                    for h in range(HPC):
                        for i in (n - 2, n - 1):
                            st, r, sel = stl[i]
                            pr = prs[i][h // 2]
                            src = pr[:, r:QW] if h % 2 == 0 \
                                else pr[:, QW:2 * QW - r]
                            nc.vector.tensor_add(dacc[h][:, r:QW],
                                                 dacc[h][:, r:QW], src)
                        finish_head(h)

            def attention_batch_general(b, sqcs):
                for sqc in sqcs:
                    sq0 = sqc * QW
                    stl = _st_info(variant, sqc)
                    n = len(stl)
                    for h in range(HPC):
                        prtiles = []
                        for st, r, sel in stl:
                            pss = ps1.tile([P, QW], FP32, name="pss", tag="ps")
                            nc.tensor.matmul(pss[:], ident[:],
                                             mk_sb[:, st, sq0:sq0 + QW],
                                             start=True, stop=False)
                            nc.tensor.matmul(
                                pss[:], k_rot[b][:, st * P:(st + 1) * P],
                                q_rot[h][b][:, sq0:sq0 + QW],
                                start=False, stop=True)
                            pr = prp.tile([P, QW], BF16, name="pr", tag="pr")
                            nc.scalar.activation(pr[:], pss[:], EXP)
                            prtiles.append(pr)
                        pso = ps2.tile([P, 2 * QW], FP32, name="psog",
                                       tag="pair")
                        for i, (st, r, sel) in enumerate(stl):
                            pr = prtiles[i]
                            nc.tensor.matmul(pso[:, 0:QW], v_sb[b][:, st, :],
                                             pr[:],
                                             start=(i == 0), stop=(i == n - 1))
                            nc.tensor.matmul(pso[:, QW:2 * QW], ones_sq[:],
                                             pr[:],
                                             start=(i == 0), stop=(i == n - 1))
                        rb = mip.tile([P, QW], FP32, name="rb")
                        nc.vector.reciprocal_approx_fast(rb[:],
                                                         pso[:, QW:2 * QW])
                        nc.vector.tensor_mul(attn[h][b][:, sq0:sq0 + QW],
                                             pso[:, 0:QW], rb[:])
                        nc.gpsimd.dma_start(
                            ag_in[b][sqc][h * P:(h + 1) * P, :],
                            attn[h][b][:, sq0:sq0 + QW])

            def gather_batch(b, c):
                nc.gpsimd.collective_compute(
                    "AllGather", mybir.AluOpType.bypass,
                    ins=[ag_in[b][c][:].opt()],
                    outs=[ag_out[b][c][:].opt()],
                    replica_groups=[list(range(NCORES))],
                )

            def wo_chunk(b, nch, last):
                """wo matmuls for one (batch, sq-chunk): full 4096-contraction
                over 512 tokens.  Processing sq-chunks serially (not
                interleaved) means chunk 0's matmuls only need that chunk's
                AllGather -- chunk 1's gather (the last collective for the
                batch) gets an extra ~35us of slack before first use."""
                ag_r = ag_out[b][nch].rearrange("(k p) t -> p k t", p=P)
                psw_pair = ps2.tile([P, 2 * QW], FP32, name="psw_pair",
                                    tag="pair")
                psw_s = [ps1.tile([P, QW], FP32, name="psw", tag="ps")
                         for _ in range(2)]

                def psw(mb):
                    if mb < 2:
                        return psw_pair[:, mb * QW:(mb + 1) * QW]
                    return psw_s[mb - 2][:]

                nchk = KT // AGC
                wtail = []
                for kc in range(nchk):
                    agt = agp.tile([P, AGC, QW], BF16, name="agt")
                    nc.sync.dma_start(agt[:],
                                      ag_r[:, kc * AGC:(kc + 1) * AGC, :])
                    if kc < nchk - 2:
                        for j in range(AGC):
                            k = kc * AGC + j
                            for mb in range(4):
                                w = wq_sb[:, k, mb * P:(mb + 1) * P]
                                nc.tensor.matmul(
                                    psw(mb), w, agt[:, j, :],
                                    start=(k == 0), stop=False)
                    else:
                        wtail += [(kc * AGC + j, agt) for j in range(AGC)]
                # last two chunks mb-major so early mb groups stop several us
                # before the end and their evictions + out DMAs overlap the
                # remaining matmuls.
                tcol = b * S + nch * QW
                for mb in range(4):
                    for k, agt in wtail:
                        w = wq_sb[:, k, mb * P:(mb + 1) * P]
                        nc.tensor.matmul(
                            psw(mb), w, agt[:, k % AGC, :],
                            start=False, stop=(k == KT - 1))
                    if mb == 3 and last:
                        # split the very last eviction in half across
                        # engines/queues so its copy + DMA pipeline
                        # instead of serializing after the final matmul.
                        hw = QW // 2
                        for hf in range(2):
                            ow = owp.tile([P, hw], BF16, name="owh",
                                          tag="owh")
                            src = psw(mb)[:, hf * hw:(hf + 1) * hw]
                            if hf == 0:
                                nc.vector.tensor_copy(ow[:], src)
                                dma = nc.sync.dma_start
                            else:
                                nc.scalar.copy(ow[:], src)
                                dma = nc.scalar.dma_start
                            dma(out_e[mb * P:(mb + 1) * P,
                                      tcol + hf * hw:tcol + (hf + 1) * hw],
                                ow[:])
                        continue
                    ow = owp.tile([P, QW], BF16, name="ow")
                    if mb % 2 == 0:
                        nc.scalar.copy(ow[:], psw(mb))
                        dma = nc.scalar.dma_start
                    else:
                        nc.vector.tensor_copy(ow[:], psw(mb))
                        dma = nc.sync.dma_start
                    dma(out_e[mb * P:(mb + 1) * P, tcol:tcol + QW], ow[:])

            def wo_batch(b):
                for nch in range(SQC):
                    wo_chunk(b, nch, last=(b == B - 1 and nch == SQC - 1))

            # ---- timeline ----
            # causal: attention sq-chunks interleave between proj quarters --
            # chunk s0 of batch b only needs that batch's first token quarter
            # (no future keys).  This fires gather(0) earlier and gives the
            # xt/weight streams HBM-quiet windows (attention phases do no HBM
            # traffic) to get ahead.  Non-causal variants attend future keys,
            # so each batch's attention must wait for BOTH its quarters.
            load_weights()
            if variant == "causal":
                proj_quarter(0)
                attention_batch(0, (0,))
                gather_batch(0, 0)
                proj_quarter(1)
                attention_batch(0, (1,))
                gather_batch(0, 1)
                proj_quarter(2)
                attention_batch(1, (0,))
                gather_batch(1, 0)
                proj_quarter(3)
                wo_load()
                attention_batch(1, (1,))
                gather_batch(1, 1)
            else:
                proj_quarter(0)
                proj_quarter(1)
                attention_batch(0, (0,))
                gather_batch(0, 0)
                attention_batch(0, (1,))
                gather_batch(0, 1)
                proj_quarter(2)
                proj_quarter(3)
                wo_load()
                attention_batch(1, (0,))
                gather_batch(1, 0)
                attention_batch(1, (1,))
                gather_batch(1, 1)
            wo_batch(0)
            wo_batch(1)

    nc.compile()
    return nc


def _get_compiled(variant):
    if variant not in _COMPILED:
        _COMPILED[variant] = _build(variant)
    return _COMPILED[variant]


def _detect_variant(mask2d):
    if not np.any(mask2d):
        return "nomask"
    tril = np.tril(mask2d)
    if not np.any(tril):
        iu = np.triu_indices(S, 1)
        if np.all(mask2d[iu] <= -1e8):
            return "causal"
    return "general"


def _pack_kt(w):
    """[R*128, N] -> [128, R, N] so that [:, k, :] is rows k*128..k*128+127."""
    return np.ascontiguousarray(w.reshape(w.shape[0] // P, P, -1).transpose(1, 0, 2))


def kernel(x, wq, wk, wv, wo, lora_q_a, lora_q_b, lora_v_a, lora_v_b,
           freqs_cos, freqs_sin, mask, start_pos=0, **_):
    global LAST_RESULTS
    bf = ml_dtypes.bfloat16
    x = np.asarray(x, np.float32)
    wq = np.asarray(wq, np.float32)
    wk = np.asarray(wk, np.float32)
    wv = np.asarray(wv, np.float32)
    wo = np.asarray(wo, np.float32)
    lora_q_a = np.asarray(lora_q_a, np.float32)
    lora_q_b = np.asarray(lora_q_b, np.float32)
    lora_v_a = np.asarray(lora_v_a, np.float32)
    lora_v_b = np.asarray(lora_v_b, np.float32)
    cos = np.asarray(freqs_cos, np.float32)
    sin = np.asarray(freqs_sin, np.float32)
    mask2d = np.asarray(mask, np.float32).reshape(S, S)

    variant = _detect_variant(mask2d)
    nc = _get_compiled(variant)

    # fold LoRA + scale; permute rope pairs (evens then odds within each head)
    wq_eff = (wq + lora_q_a @ lora_q_b) * np.float32(1.0 / np.sqrt(HD))
    wv_eff = wv + lora_v_a @ lora_v_b
    perm = np.concatenate([np.arange(0, HD, 2), np.arange(1, HD, 2)])
    qperm = (np.arange(H)[:, None] * HD + perm[None, :]).reshape(-1)
    kperm = (np.arange(KVH)[:, None] * HD + perm[None, :]).reshape(-1)
    wq_eff = wq_eff[:, qperm]
    wk_p = wk[:, kperm]

    xt = np.ascontiguousarray(x.reshape(T, D).T)        # [4096, 2048]
    # [128, KT, T] -> quarter-major [128, NQ, KT, QW] (contiguous per chunk)
    xt_p = np.ascontiguousarray(
        _pack_kt(xt).reshape(P, KT, NQ, QW).transpose(0, 2, 1, 3)).astype(bf)
    c64 = np.tile(cos.T, (1, B))                        # [64, 2048]
    s64 = np.tile(sin.T, (1, B))
    cosT = np.concatenate([c64, c64], axis=0).astype(bf)   # [c; c]
    sinT = np.concatenate([s64, -s64], axis=0).astype(bf)  # [s; -s]

    if variant == "general":
        maskT = np.ascontiguousarray(mask2d.T)          # [st, sq]
        mk = _pack_kt(maskT).astype(bf)                 # [128, 8, 1024]
    else:
        mk = None

    in_maps = []
    for c in range(NCORES):
        im = {
            "xt": xt_p,
            "wq": _pack_kt(wq_eff[:, c * QCOLS:(c + 1) * QCOLS]).astype(bf),
            "wk": _pack_kt(wk_p[:, c * HD:(c + 1) * HD]).astype(bf),
            "wv": _pack_kt(wv_eff[:, c * HD:(c + 1) * HD]).astype(bf),
            "wo": _pack_kt(wo[:, c * QCOLS:(c + 1) * QCOLS]).astype(bf),
            "cos": cosT,
            "sin": sinT,
        }
        if mk is not None:
            im["mk"] = mk
        in_maps.append(im)

    res = run_bass_kernel_spmd(nc, in_maps, core_ids=list(range(NCORES)))
    LAST_RESULTS = res
    outT = np.concatenate([res.results[c]["out"] for c in range(NCORES)], axis=0)
    return np.ascontiguousarray(outT.T).reshape(B, S, D).astype(np.float32)
